# revision 1
# baseline (speedup 1.0000x reference)
"""DeepseekV3 MLA attention (B=1, S=2048, D=2048, H=16) on 8 trn2 NeuronCores.

Strategy (tensor-parallel over heads, replicated low-rank projections):
  - every core computes the full q_a / kv_a low-rank projections (+rmsnorm)
    from a host-transposed hidden state, entirely in a "transposed" layout
    (feature dim on partitions, sequence on the free dim) so attention
    operands come out pre-transposed for the PE;
  - each core owns 2 heads: it computes q_b / kv_b for them, causal
    flash-style attention (no max subtraction -- logits are O(1) here), and
    its slice of o_proj, producing a partial [S, D] output;
  - host sums the 8 partials.

All matmuls run in bf16 (fp32 PSUM accumulation); rmsnorm stats, rope and
softmax run in fp32.  RoPE deinterleave + rotate-half are folded into the
weight layout on the host (extra "pre-swapped, sign-folded" weight columns)
so the device only does aligned elementwise mul/adds.
"""

import numpy as np
import ml_dtypes

import concourse.bass as bass
import concourse.mybir as mybir
import concourse.tile as tile
from concourse.bass_utils import run_bass_kernel_spmd

BF16 = ml_dtypes.bfloat16
F32 = mybir.dt.float32
BF = mybir.dt.bfloat16

B, S, D = 1, 2048, 2048
H = 16
N_CORES = 8
HPC = H // N_CORES  # heads per core = 2
Q_LORA = 1536
KV_LORA = 512
NOPE = 128
ROPE = 64
VD = 128
QHD = NOPE + ROPE  # 192
THETA = 50000.0
EPS = 1e-6
SCALE = QHD ** (-0.5)

NQ = 512            # q-chunk (matmul free dim)
NCHUNK = S // NQ    # 4
KT = S // 128       # 16 k-tiles
AF = mybir.ActivationFunctionType

LAST_RESULTS = None
_CACHE = {}


# ----------------------------------------------------------------------------
# host-side weight preparation
# ----------------------------------------------------------------------------

def _deint_perm():
    # deinterleave: out[j] = in[2j] (j<32), in[2(j-32)+1] (j>=32)
    p = np.empty(ROPE, dtype=np.int64)
    p[:32] = 2 * np.arange(32)
    p[32:] = 2 * np.arange(32) + 1
    return p


def _rope_tables(position_ids):
    pos = np.asarray(position_ids).reshape(-1).astype(np.float32)  # [S]
    inv_freq = (1.0 / (THETA ** (np.arange(0, ROPE, 2, dtype=np.float32) / ROPE)))
    freqs = np.outer(pos, inv_freq)  # [S, 32]
    cos32 = np.cos(freqs).T.astype(np.float32)  # [32, S]
    sin32 = np.sin(freqs).T.astype(np.float32)
    cos128 = np.tile(cos32, (4, 1))  # [128, S]
    sin128 = np.tile(sin32, (4, 1))
    return cos128, sin128


def _causal_mask_big():
    # M[dk, u] = 1 if u >= dk + 384 ; slice [:, 384-128*i : 896-128*i]
    # gives the diagonal-block mask indicator(dq >= dk + 128*i)
    dk = np.arange(128)[:, None]
    u = np.arange(1024)[None, :]
    return (u >= dk + 384).astype(BF16)


def _prep_inputs(inputs):
    hidden = np.asarray(inputs["hidden_states"], dtype=np.float32)[0]  # [S, D]
    position_ids = np.asarray(inputs["position_ids"])
    q_a_w = np.asarray(inputs["q_a_w"], dtype=np.float32)        # [1536, D]
    q_a_ln_w = np.asarray(inputs["q_a_ln_w"], dtype=np.float32)  # [1536]
    q_b_w = np.asarray(inputs["q_b_w"], dtype=np.float32)        # [H*192, 1536]
    kv_a_w = np.asarray(inputs["kv_a_w"], dtype=np.float32)      # [576, D]
    kv_a_ln_w = np.asarray(inputs["kv_a_ln_w"], dtype=np.float32)  # [512]
    kv_b_w = np.asarray(inputs["kv_b_w"], dtype=np.float32)      # [H*256, 512]
    o_w = np.asarray(inputs["o_w"], dtype=np.float32)            # [D, H*128]

    dp = _deint_perm()
    dps = dp[(np.arange(ROPE) ^ 32)]          # source index for the swapped term
    sgn = np.where(np.arange(ROPE) < 32, -1.0, 1.0).astype(np.float32)[:, None]

    shared = {}
    shared["hT"] = np.ascontiguousarray(hidden.T).astype(BF16)          # [D, S]
    shared["qaT"] = np.ascontiguousarray(q_a_w.T).astype(BF16)          # [D, 1536]

    # kv_a columns: [ckv 512 | kpe 64 (deint) | kpe2 64 (swap+sign)]
    kva_cols = np.concatenate(
        [kv_a_w[:KV_LORA], kv_a_w[KV_LORA + dp], sgn * kv_a_w[KV_LORA + dps]], axis=0
    )  # [640, D]
    shared["kvaT"] = np.ascontiguousarray(kva_cols.T).astype(BF16)      # [D, 640]

    cos128, sin128 = _rope_tables(position_ids)
    shared["cosb"] = cos128
    shared["sinb"] = sin128
    shared["maskb"] = _causal_mask_big()

    # q_b with ln + scale folded
    qb = q_b_w * q_a_ln_w[None, :] * SCALE  # [H*192, 1536]
    qb = qb.reshape(H, QHD, Q_LORA)
    kvb = (kv_b_w * kv_a_ln_w[None, :]).reshape(H, NOPE + VD, KV_LORA)

    per_core = []
    for c in range(N_CORES):
        h0, h1 = HPC * c, HPC * c + 1
        nope0 = qb[h0, :NOPE]            # [128, 1536]
        nope1 = qb[h1, :NOPE]
        peP = np.concatenate([qb[h0, NOPE + dp], qb[h1, NOPE + dp]], axis=0)  # [128,...]
        pe2P = np.concatenate(
            [sgn * qb[h0, NOPE + dps], sgn * qb[h1, NOPE + dps]], axis=0
        )
        qb_cols = np.concatenate([nope0, nope1, peP, pe2P], axis=0)  # [512, 1536]
        kb_cols = np.concatenate([kvb[h0, :NOPE], kvb[h1, :NOPE]], axis=0)  # [256, 512]
        vb_cols = np.concatenate([kvb[h0, NOPE:], kvb[h1, NOPE:]], axis=0)  # [256, 512]
        o_slice = o_w[:, VD * h0 : VD * (h1 + 1)]  # [D, 256]
        per_core.append(
            {
                "qbT": np.ascontiguousarray(qb_cols.T).astype(BF16),   # [1536, 512]
                "kbT": np.ascontiguousarray(kb_cols.T).astype(BF16),   # [512, 256]
                "vbT": np.ascontiguousarray(vb_cols.T).astype(BF16),   # [512, 256]
                "owT": np.ascontiguousarray(o_slice.T).astype(BF16),   # [256, S... D]
            }
        )
    return shared, per_core


# ----------------------------------------------------------------------------
# numpy simulation of the device program (for host-side validation)
# ----------------------------------------------------------------------------

def _sim_core(shared, pc):
    bf = lambda x: x.astype(BF16).astype(np.float32)
    hT = shared["hT"].astype(np.float32)          # [D, S]
    qaT = shared["qaT"].astype(np.float32)        # [D, 1536]
    kvaT = shared["kvaT"].astype(np.float32)      # [D, 640]
    cos = shared["cosb"]                          # [128, S]
    sin = shared["sinb"]
    qbT = pc["qbT"].astype(np.float32)            # [1536, 512]
    kbT = pc["kbT"].astype(np.float32)            # [512, 256]
    vbT = pc["vbT"].astype(np.float32)            # [512, 256]
    owT = pc["owT"].astype(np.float32)            # [256, D]

    qaTx = qaT.T @ hT                             # [1536, S]
    qaTb = bf(qaTx)                               # bf16 copy used downstream
    ssq = (bf(qaTb * qaTb)).sum(axis=0)           # square in bf16, fp32 sum
    inv = 1.0 / np.sqrt(ssq / Q_LORA + EPS)       # [S]
    qT = qbT.T @ qaTb                             # [512, S]
    qn0 = bf(qT[0:128] * inv)
    qn1 = bf(qT[128:256] * inv)
    pe, pe2 = qT[256:384], qT[384:512]
    qpe = bf((pe * cos + pe2 * sin) * inv)        # [128, S] packed (h0;h1)

    ckvT = kvaT.T @ hT                            # [640, S]
    ckv = ckvT[:KV_LORA]
    ckvb = bf(ckv)
    ssc = (bf(ckvb * ckvb)).sum(axis=0)
    invc = 1.0 / np.sqrt(ssc / KV_LORA + EPS)
    ckvn = bf(ckvb * invc)                        # [512, S]
    kpe, kpe2 = ckvT[512:576], ckvT[576:640]
    kper = bf(kpe * cos[0:64] + kpe2 * sin[0:64])  # [64, S]

    out = np.zeros((S, D), dtype=np.float32)
    for j in range(HPC):
        knT = bf(kbT[:, 128 * j : 128 * (j + 1)].T @ ckvn)   # [128, S]
        v = bf(ckvn.T @ vbT[:, 128 * j : 128 * (j + 1)])     # [S, 128]
        qn = qn0 if j == 0 else qn1
        qp = qpe[64 * j : 64 * (j + 1)]
        scores = knT.T @ qn + kper.T @ qp         # [S(k), S(q)] -> st[k, q]
        st = scores
        kidx = np.arange(S)[:, None]
        qidx = np.arange(S)[None, :]
        p = np.exp(st) * (kidx <= qidx)
        p = bf(p)
        rs = p.sum(axis=0)                        # [q]
        oT = (v.T @ p)                            # [128, q]
        oT = bf(oT * (1.0 / rs))
        out += oT.T @ owT[128 * j : 128 * (j + 1)]
    return out


def sim(inputs):
    shared, per_core = _prep_inputs(inputs)
    out = np.zeros((S, D), dtype=np.float32)
    for c in range(N_CORES):
        out += _sim_core(shared, per_core[c])
    return out.reshape(B, S, D)


# ----------------------------------------------------------------------------
# bass program
# ----------------------------------------------------------------------------

def _split_waits(nc, max_waits=1):
    """This walrus build accepts at most one sem wait per instruction; hoist
    excess waits onto pure-wait EventSemaphore carriers just before it."""
    n_new = 0
    for f in nc.m.functions:
        for blk in f.blocks:
            new_insts = []
            for inst in blk.instructions:
                si = getattr(inst, "sync_info", None)
                waits = list(si.on_wait) if (si is not None and si.on_wait) else []
                if len(waits) > max_waits:
                    extra, keep = waits[:-max_waits], waits[-max_waits:]
                    for w in extra:
                        n_new += 1
                        carrier = mybir.InstEventSemaphore(
                            name=f"ws-{n_new}-{inst.name}",
                            engine=inst.engine,
                            ins=[],
                            outs=[],
                            sync_info=mybir.SyncInfo(on_wait=[w], on_update=[]),
                        )
                        nc.register_instruction(carrier, overwrite=True)
                        new_insts.append(carrier)
                    si.on_wait = keep
                new_insts.append(inst)
            blk.instructions = new_insts
    return n_new


def _build_nc():
    nc = bass.Bass()
    hT = nc.dram_tensor("hT", [D, S], BF, kind="ExternalInput")
    qaT = nc.dram_tensor("qaT", [D, Q_LORA], BF, kind="ExternalInput")
    kvaT = nc.dram_tensor("kvaT", [D, 640], BF, kind="ExternalInput")
    qbT = nc.dram_tensor("qbT", [Q_LORA, 512], BF, kind="ExternalInput")
    kbT = nc.dram_tensor("kbT", [KV_LORA, 256], BF, kind="ExternalInput")
    vbT = nc.dram_tensor("vbT", [KV_LORA, 256], BF, kind="ExternalInput")
    owT = nc.dram_tensor("owT", [2 * VD, D], BF, kind="ExternalInput")
    cosb = nc.dram_tensor("cosb", [128, S], F32, kind="ExternalInput")
    sinb = nc.dram_tensor("sinb", [128, S], F32, kind="ExternalInput")
    maskb = nc.dram_tensor("maskb", [128, 1024], BF, kind="ExternalInput")
    out = nc.dram_tensor("out", [S, D], F32, kind="ExternalOutput")

    QL_T = Q_LORA // 128  # 12
    D_T = D // 128        # 16
    CV_T = KV_LORA // 128  # 4

    with tile.TileContext(nc) as tc:
        with tc.tile_pool(name="persist1", bufs=1) as persist1:
            ones_t = persist1.tile([128, 128], BF, tag="ones")
            eps_t = persist1.tile([128, 1], F32, tag="eps")
            nc.vector.memset(eps_t, EPS)
            nc.vector.memset(ones_t, 1.0)
            qn_T = [persist1.tile([128, S], BF, tag=f"qnT{h}", name=f"qnT{h}") for h in range(HPC)]
            qpeP = persist1.tile([128, S], BF, tag="qpeP")
            qpe1 = persist1.tile([64, S], BF, tag="qpe1")
            ckvn = [persist1.tile([128, S], BF, tag=f"ckvn{i}", name=f"ckvn{i}") for i in range(CV_T)]
            kperLo = persist1.tile([128, S], BF, tag="kperLo")
            kperHi = persist1.tile([128, S], BF, tag="kperHi")
            nc.vector.memset(kperLo[64:128, :], 0.0)
            nc.vector.memset(kperHi[0:64, :], 0.0)

            # ------------- merged stage 1: q & kv paths, one hidden pass -------------
            with tc.tile_pool(name="qaw", bufs=1) as qaw, \
                 tc.tile_pool(name="kvw", bufs=1) as kvw, \
                 tc.tile_pool(name="qbw", bufs=1) as qbw, \
                 tc.tile_pool(name="hx", bufs=2) as hx, \
                 tc.tile_pool(name="qasb", bufs=1) as qasb, \
                 tc.tile_pool(name="cvsb", bufs=1) as cvsb, \
                 tc.tile_pool(name="csp", bufs=2) as csp, \
                 tc.tile_pool(name="sq", bufs=2) as sqp, \
                 tc.tile_pool(name="nrm", bufs=2) as nrm, \
                 tc.tile_pool(name="nrm2", bufs=2) as nrm2, \
                 tc.tile_pool(name="pet", bufs=1) as pet, \
                 tc.tile_pool(name="st_ps", bufs=3, space="PSUM") as st_ps, \
                 tc.tile_pool(name="ssq_ps", bufs=1, space="PSUM") as ssq_ps, \
                 tc.tile_pool(name="ssq2_ps", bufs=1, space="PSUM") as ssq2_ps, \
                 tc.tile_pool(name="qt_ps", bufs=3, space="PSUM") as qt_ps:

                qa_w = qaw.tile([128, D_T, Q_LORA], BF, tag="qaw")
                kva_w = kvw.tile([128, D_T, 640], BF, tag="kvw")
                qb_w = qbw.tile([128, QL_T, 512], BF, tag="qbw")
                for k in range(D_T):
                    nc.sync.dma_start(out=kva_w[:, k, :], in_=kvaT[128 * k : 128 * (k + 1), :])

                for c in range(NCHUNK):
                    cs = slice(NQ * c, NQ * (c + 1))
                    h_t = hx.tile([128, D_T, NQ], BF, tag="h")
                    for k in range(D_T):
                        nc.sync.dma_start(out=h_t[:, k, :], in_=hT[128 * k : 128 * (k + 1), cs])
                    cos_c = csp.tile([128, NQ], F32, tag="cosc")
                    sin_c = csp.tile([128, NQ], F32, tag="sinc")
                    nc.sync.dma_start(out=cos_c, in_=cosb[:, cs])
                    nc.sync.dma_start(out=sin_c, in_=sinb[:, cs])
                    if c == 0:
                        for k in range(D_T):
                            nc.sync.dma_start(out=qa_w[:, k, :], in_=qaT[128 * k : 128 * (k + 1), :])
                        for m in range(QL_T):
                            nc.sync.dma_start(out=qb_w[:, m, :], in_=qbT[128 * m : 128 * (m + 1), :])

                    # ---- kv_a: 4 ckv m-tiles + kpe + kpe2 ----
                    cv_t = cvsb.tile([128, CV_T, NQ], BF, tag="cv")
                    ssc = ssq2_ps.tile([128, NQ], F32, tag="ssc")
                    pe_ps = []
                    for m in range(6):
                        mp = 128 if m < 4 else 64
                        col = slice(128 * m, 128 * m + 128) if m < 4 else \
                            slice(512 + 64 * (m - 4), 512 + 64 * (m - 3))
                        ps = st_ps.tile([mp, NQ], F32, tag="stps")
                        for k in range(D_T):
                            nc.tensor.matmul(
                                ps,
                                kva_w[:, k, col],
                                h_t[:, k, :],
                                start=(k == 0),
                                stop=(k == D_T - 1),
                            )
                        if m < 4:
                            nc.vector.tensor_copy(cv_t[:, m, :], ps)
                            sq = sqp.tile([128, NQ], BF, tag="sq")
                            nc.scalar.activation(out=sq, in_=ps, func=AF.Square)
                            nc.tensor.matmul(
                                ssc, ones_t, sq, start=(m == 0), stop=(m == CV_T - 1)
                            )
                        else:
                            pe_ps.append(ps)

                    # ---- q_a: 12 m-tiles ----
                    qa_t = qasb.tile([128, QL_T, NQ], BF, tag="qa")
                    ssq = ssq_ps.tile([128, NQ], F32, tag="ssq")
                    for m in range(QL_T):
                        ps = st_ps.tile([128, NQ], F32, tag="stps")
                        for k in range(D_T):
                            nc.tensor.matmul(
                                ps,
                                qa_w[:, k, 128 * m : 128 * (m + 1)],
                                h_t[:, k, :],
                                start=(k == 0),
                                stop=(k == D_T - 1),
                            )
                        nc.vector.tensor_copy(qa_t[:, m, :], ps)
                        sq = sqp.tile([128, NQ], BF, tag="sq")
                        nc.scalar.activation(out=sq, in_=ps, func=AF.Square)
                        nc.tensor.matmul(
                            ssq, ones_t, sq, start=(m == 0), stop=(m == QL_T - 1)
                        )

                    # ---- kv norm + kpe rope ----
                    bc2 = nrm2.tile([128, NQ], F32, tag="bc2")
                    nc.scalar.activation(
                        out=bc2, in_=ssc, func=AF.Sqrt, scale=1.0 / KV_LORA, bias=eps_t
                    )
                    nc.vector.reciprocal(bc2, bc2)
                    for i in range(CV_T):
                        nc.vector.tensor_mul(ckvn[i][:, cs], cv_t[:, i, :], bc2)
                    t1 = pet.tile([128, NQ], F32, tag="t1")
                    t2 = pet.tile([128, NQ], F32, tag="t2")
                    nc.vector.tensor_mul(t1[0:64, :], pe_ps[0], cos_c[0:64, :])
                    nc.vector.tensor_mul(t2[0:64, :], pe_ps[1], sin_c[0:64, :])
                    nc.vector.tensor_add(kperLo[0:64, cs], t1[0:64, :], t2[0:64, :])
                    nc.vector.tensor_add(kperHi[64:128, cs], t1[0:64, :], t2[0:64, :])

                    # ---- q_b: 4 col-blocks accumulated over 12 m ----
                    bc = nrm.tile([128, NQ], F32, tag="bc")
                    nc.scalar.activation(
                        out=bc, in_=ssq, func=AF.Sqrt, scale=1.0 / Q_LORA, bias=eps_t
                    )
                    nc.vector.reciprocal(bc, bc)
                    qt_tiles = []
                    for b in range(4):
                        ps = qt_ps.tile([128, NQ], F32, tag="qtps")
                        for m in range(QL_T):
                            nc.tensor.matmul(
                                ps,
                                qb_w[:, m, 128 * b : 128 * (b + 1)],
                                qa_t[:, m, :],
                                start=(m == 0),
                                stop=(m == QL_T - 1),
                            )
                        if b == 0:
                            nc.vector.tensor_mul(qn_T[0][:, cs], ps, bc)
                        elif b == 1:
                            nc.vector.tensor_mul(qn_T[1][:, cs], ps, bc)
                        else:
                            qt_tiles.append(ps)
                    nc.vector.tensor_mul(t1, qt_tiles[0], cos_c)
                    nc.vector.tensor_mul(t2, qt_tiles[1], sin_c)
                    nc.vector.tensor_add(t1, t1, t2)
                    nc.vector.tensor_mul(qpeP[:, cs], t1, bc)
                nc.sync.dma_start(out=qpe1[:, :], in_=qpeP[64:128, :])

            # ---------------- phase B2: kv_b projections ----------------
            with tc.tile_pool(name="persist2", bufs=1) as persist2:
                kn_T = [persist2.tile([128, S], BF, tag=f"knT{h}", name=f"knT{h}") for h in range(HPC)]
                v_sb = [persist2.tile([128, S], BF, tag=f"v{h}", name=f"v{h}") for h in range(HPC)]
                o_T = [persist2.tile([128, S], BF, tag=f"oT{h}", name=f"oT{h}") for h in range(HPC)]
                with tc.tile_pool(name="kbw", bufs=1) as kbw, \
                     tc.tile_pool(name="kn_ps", bufs=2, space="PSUM") as kn_ps, \
                     tc.tile_pool(name="v_ps", bufs=3, space="PSUM") as v_ps:
                    kb_w = kbw.tile([128, CV_T, 256], BF, tag="kbw")
                    vb_w = kbw.tile([128, CV_T, 256], BF, tag="vbw")
                    for ct in range(CV_T):
                        nc.sync.dma_start(out=kb_w[:, ct, :], in_=kbT[128 * ct : 128 * (ct + 1), :])
                        nc.sync.dma_start(out=vb_w[:, ct, :], in_=vbT[128 * ct : 128 * (ct + 1), :])
                    for h in range(HPC):
                        hs = slice(128 * h, 128 * (h + 1))
                        for c in range(NCHUNK):
                            cs = slice(NQ * c, NQ * (c + 1))
                            ps = kn_ps.tile([128, NQ], F32, tag="knps")
                            for ct in range(CV_T):
                                nc.tensor.matmul(
                                    ps,
                                    kb_w[:, ct, hs],
                                    ckvn[ct][:, cs],
                                    start=(ct == 0),
                                    stop=(ct == CV_T - 1),
                                )
                            nc.vector.tensor_copy(kn_T[h][:, cs], ps)
                        for kt in range(KT):
                            ks = slice(128 * kt, 128 * (kt + 1))
                            ps = v_ps.tile([128, VD], F32, tag="vps")
                            for ct in range(CV_T):
                                nc.tensor.matmul(
                                    ps,
                                    ckvn[ct][:, ks],
                                    vb_w[:, ct, hs],
                                    start=(ct == 0),
                                    stop=(ct == CV_T - 1),
                                )
                            nc.vector.tensor_copy(v_sb[h][:, ks], ps)

                # ---------------- phase C: attention ----------------
                mskp_cm = tc.tile_pool(name="mskp", bufs=1)
                oww_cm = tc.tile_pool(name="oww", bufs=1)
                mskp = mskp_cm.__enter__()
                oww = oww_cm.__enter__()
                with tc.tile_pool(name="pp", bufs=6) as pp, \
                     tc.tile_pool(name="ep", bufs=3) as ep, \
                     tc.tile_pool(name="rvp", bufs=2) as rvp, \
                     tc.tile_pool(name="ostg", bufs=4) as ostg, \
                     tc.tile_pool(name="s_ps", bufs=3, space="PSUM") as s_ps, \
                     tc.tile_pool(name="rs_ps", bufs=2, space="PSUM") as rs_ps, \
                     tc.tile_pool(name="o_ps", bufs=2, space="PSUM") as o_ps, \
                     tc.tile_pool(name="out_ps", bufs=1, space="PSUM") as out_ps:
                    mask_s = mskp.tile([128, 1024], BF, tag="mask")
                    nc.sync.dma_start(out=mask_s, in_=maskb[:, :])
                    ow_t = oww.tile([128, HPC, D], BF, tag="oww")
                    for j in range(HPC):
                        nc.sync.dma_start(out=ow_t[:, j, :], in_=owT[128 * j : 128 * (j + 1), :])
                    for c in range(NCHUNK):
                        cs = slice(NQ * c, NQ * (c + 1))
                        nkt = 4 * (c + 1)
                        for h in range(HPC):
                            kper_h = kperLo if h == 0 else kperHi
                            rs = rs_ps.tile([128, NQ], F32, tag="rs")
                            op = o_ps.tile([128, NQ], F32, tag="op")
                            for kt in range(nkt):
                                ks = slice(128 * kt, 128 * (kt + 1))
                                i = kt - 4 * c
                                lo = 128 * i if i > 0 else 0  # valid q-subrange start
                                qs = slice(NQ * c + lo, NQ * (c + 1))
                                vs = slice(lo, NQ)
                                sp = s_ps.tile([128, NQ], F32, tag="sp")
                                nc.tensor.matmul(
                                    sp[:, vs], kn_T[h][:, ks], qn_T[h][:, qs],
                                    start=True, stop=False,
                                )
                                nc.tensor.matmul(
                                    sp[:, vs], kper_h[:, ks], qpeP[:, qs],
                                    start=False, stop=True,
                                )
                                p_t = pp.tile([128, NQ], BF, tag="p")
                                if kt >= 4 * c:
                                    e_t = ep.tile([128, NQ], BF, tag="e")
                                    nc.scalar.activation(out=e_t[:, vs], in_=sp[:, vs], func=AF.Exp)
                                    nc.vector.tensor_mul(
                                        p_t[:, vs], e_t[:, vs],
                                        mask_s[:, 384 : 896 - lo],
                                    )
                                else:
                                    nc.scalar.activation(out=p_t[:, vs], in_=sp[:, vs], func=AF.Exp)
                                nc.tensor.matmul(
                                    rs[:, vs], ones_t, p_t[:, vs],
                                    start=(kt == 0), stop=(kt == nkt - 1),
                                )
                                nc.tensor.matmul(
                                    op[:, vs],
                                    v_sb[h][:, ks],
                                    p_t[:, vs],
                                    start=(kt == 0), stop=(kt == nkt - 1),
                                )
                            rv = rvp.tile([128, NQ], F32, tag="rv")
                            nc.vector.reciprocal(rv, rs)
                            nc.vector.tensor_mul(o_T[h][:, cs], op, rv)
                        # o_proj for this chunk's 4 s-tiles (both heads now done;
                        # last chunk handled in a post-phase with deeper PSUM)
                        for si in range(4 * c, 4 * (c + 1) if c < NCHUNK - 1 else 4 * c):
                            ss = slice(128 * si, 128 * (si + 1))
                            for nch in range(NCHUNK):
                                ns = slice(NQ * nch, NQ * (nch + 1))
                                ps = out_ps.tile([128, NQ], F32, tag="outps")
                                for j in range(HPC):
                                    nc.tensor.matmul(
                                        ps,
                                        o_T[j][:, ss],
                                        ow_t[:, j, ns],
                                        start=(j == 0),
                                        stop=(j == HPC - 1),
                                    )
                                stg = ostg.tile([128, NQ], F32, tag="ostg")
                                nc.scalar.activation(out=stg, in_=ps, func=AF.Copy)
                                nc.sync.dma_start(out=out[ss, ns], in_=stg)
                # ---------------- final chunk o_proj ----------------
                with tc.tile_pool(name="ostg2", bufs=4) as ostg2, \
                     tc.tile_pool(name="out2_ps", bufs=4, space="PSUM") as out2_ps:
                    for si in range(4 * (NCHUNK - 1), 4 * NCHUNK):
                        ss = slice(128 * si, 128 * (si + 1))
                        for nch in range(NCHUNK):
                            ns = slice(NQ * nch, NQ * (nch + 1))
                            ps = out2_ps.tile([128, NQ], F32, tag="out2ps")
                            for j in range(HPC):
                                nc.tensor.matmul(
                                    ps,
                                    o_T[j][:, ss],
                                    ow_t[:, j, ns],
                                    start=(j == 0),
                                    stop=(j == HPC - 1),
                                )
                            stg = ostg2.tile([128, NQ], F32, tag="ostg2")
                            nc.scalar.activation(out=stg, in_=ps, func=AF.Copy)
                            nc.sync.dma_start(out=out[ss, ns], in_=stg)
                oww_cm.__exit__(None, None, None)
                mskp_cm.__exit__(None, None, None)

    _split_waits(nc)
    return nc


# ----------------------------------------------------------------------------
# entry point
# ----------------------------------------------------------------------------

def kernel(**inputs):
    global LAST_RESULTS
    shared, per_core = _prep_inputs(inputs)
    if "nc" not in _CACHE:
        _CACHE["nc"] = _build_nc()
    nc = _CACHE["nc"]
    in_maps = []
    for c in range(N_CORES):
        m = {
            "hT": shared["hT"],
            "qaT": shared["qaT"],
            "kvaT": shared["kvaT"],
            "cosb": shared["cosb"],
            "sinb": shared["sinb"],
            "maskb": shared["maskb"],
            "qbT": per_core[c]["qbT"],
            "kbT": per_core[c]["kbT"],
            "vbT": per_core[c]["vbT"],
            "owT": per_core[c]["owT"],
        }
        in_maps.append(m)
    res = run_bass_kernel_spmd(nc, in_maps, core_ids=list(range(N_CORES)))
    LAST_RESULTS = res
    out = np.zeros((S, D), dtype=np.float32)
    for r in res.results:
        out += r["out"]
    return out.reshape(B, S, D)



# revision 3
# speedup vs baseline: 1.3297x; 1.3297x over previous
"""DeepseekV3 MLA attention (B=1, S=2048, D=2048, H=16) on 8 trn2 NeuronCores.

Strategy (v2: tensor-parallel heads + sharded shared projections + on-device
AllGather):
  - the q_b@q_a product is FUSED on the host per core's 2 heads, so each core
    computes q directly from hidden states (contraction D=2048) and nobody
    materializes the full q_a activation;
  - the q rmsnorm statistic is sharded: core c computes q_a only for seq
    positions [256c, 256c+256) (all 1536 features), reduces to a 1/rms row,
    and an 8-core AllGather ([1,256] f32 per core) distributes it;
  - kv_a is sharded the same way: core c computes ckv for its 256 positions,
    normalizes + ropes locally, and an AllGather ([576,256] bf16 per core)
    distributes the shared K/V latents to every core;
  - each core owns 2 heads: kv_b, causal flash-style attention (no max
    subtraction -- logits are O(1) here), and its slice of o_proj, producing
    a partial [S, D] output; host sums the 8 partials.

All matmuls run in bf16 (fp32 PSUM accumulation); rmsnorm stats, rope and
softmax run in fp32.  RoPE deinterleave + rotate-half are folded into the
weight layout on the host (extra "pre-swapped, sign-folded" weight columns)
so the device only does aligned elementwise mul/adds.
"""

import numpy as np
import ml_dtypes

import concourse.bass as bass
import concourse.mybir as mybir
import concourse.tile as tile
from concourse.bass_utils import run_bass_kernel_spmd

BF16 = ml_dtypes.bfloat16
F32 = mybir.dt.float32
BF = mybir.dt.bfloat16

B, S, D = 1, 2048, 2048
H = 16
N_CORES = 8
HPC = H // N_CORES  # heads per core = 2
Q_LORA = 1536
KV_LORA = 512
NOPE = 128
ROPE = 64
VD = 128
QHD = NOPE + ROPE  # 192
THETA = 50000.0
EPS = 1e-6
SCALE = QHD ** (-0.5)

NQ = 512            # q-chunk (matmul free dim)
NCHUNK = S // NQ    # 4
KT = S // 128       # 16 k-tiles
SB = S // N_CORES   # 256: per-core seq block for the sharded projections
AF = mybir.ActivationFunctionType

LAST_RESULTS = None
_CACHE = {}


# ----------------------------------------------------------------------------
# host-side weight preparation
# ----------------------------------------------------------------------------

def _deint_perm():
    # deinterleave: out[j] = in[2j] (j<32), in[2(j-32)+1] (j>=32)
    p = np.empty(ROPE, dtype=np.int64)
    p[:32] = 2 * np.arange(32)
    p[32:] = 2 * np.arange(32) + 1
    return p


def _rope_tables(position_ids):
    pos = np.asarray(position_ids).reshape(-1).astype(np.float32)  # [S]
    inv_freq = (1.0 / (THETA ** (np.arange(0, ROPE, 2, dtype=np.float32) / ROPE)))
    freqs = np.outer(pos, inv_freq)  # [S, 32]
    cos32 = np.cos(freqs).T.astype(np.float32)  # [32, S]
    sin32 = np.sin(freqs).T.astype(np.float32)
    cos128 = np.tile(cos32, (4, 1))  # [128, S]
    sin128 = np.tile(sin32, (4, 1))
    return cos128, sin128


def _causal_mask_big():
    # M[dk, u] = 1 if u >= dk + 384 ; slice [:, 384-128*i : 896-128*i]
    # gives the diagonal-block mask indicator(dq >= dk + 128*i)
    dk = np.arange(128)[:, None]
    u = np.arange(1024)[None, :]
    return (u >= dk + 384).astype(BF16)


def _prep_inputs(inputs):
    hidden = np.asarray(inputs["hidden_states"], dtype=np.float32)[0]  # [S, D]
    position_ids = np.asarray(inputs["position_ids"])
    q_a_w = np.asarray(inputs["q_a_w"], dtype=np.float32)        # [1536, D]
    q_a_ln_w = np.asarray(inputs["q_a_ln_w"], dtype=np.float32)  # [1536]
    q_b_w = np.asarray(inputs["q_b_w"], dtype=np.float32)        # [H*192, 1536]
    kv_a_w = np.asarray(inputs["kv_a_w"], dtype=np.float32)      # [576, D]
    kv_a_ln_w = np.asarray(inputs["kv_a_ln_w"], dtype=np.float32)  # [512]
    kv_b_w = np.asarray(inputs["kv_b_w"], dtype=np.float32)      # [H*256, 512]
    o_w = np.asarray(inputs["o_w"], dtype=np.float32)            # [D, H*128]

    dp = _deint_perm()
    dps = dp[(np.arange(ROPE) ^ 32)]          # source index for the swapped term
    sgn = np.where(np.arange(ROPE) < 32, -1.0, 1.0).astype(np.float32)[:, None]

    shared = {}
    shared["hT"] = np.ascontiguousarray(hidden.T).astype(BF16)          # [D, S]
    shared["qaT"] = np.ascontiguousarray(q_a_w.T).astype(BF16)          # [D, 1536]

    # kv_a columns: [ckv 512 | kpe 64 (deint) | kpe2 64 (swap+sign)]
    kva_cols = np.concatenate(
        [kv_a_w[:KV_LORA], kv_a_w[KV_LORA + dp], sgn * kv_a_w[KV_LORA + dps]], axis=0
    )  # [640, D]
    shared["kvaT"] = np.ascontiguousarray(kva_cols.T).astype(BF16)      # [D, 640]

    cos128, sin128 = _rope_tables(position_ids)
    shared["cosb"] = cos128
    shared["sinb"] = sin128
    shared["maskb"] = _causal_mask_big()

    # q_b with ln + scale folded
    qb = q_b_w * q_a_ln_w[None, :] * SCALE  # [H*192, 1536]
    qb = qb.reshape(H, QHD, Q_LORA)
    kvb = (kv_b_w * kv_a_ln_w[None, :]).reshape(H, NOPE + VD, KV_LORA)

    per_core = []
    for c in range(N_CORES):
        h0, h1 = HPC * c, HPC * c + 1
        nope0 = qb[h0, :NOPE]            # [128, 1536]
        nope1 = qb[h1, :NOPE]
        peP = np.concatenate([qb[h0, NOPE + dp], qb[h1, NOPE + dp]], axis=0)  # [128,...]
        pe2P = np.concatenate(
            [sgn * qb[h0, NOPE + dps], sgn * qb[h1, NOPE + dps]], axis=0
        )
        qb_cols = np.concatenate([nope0, nope1, peP, pe2P], axis=0)  # [512, 1536]
        # fuse q_b @ q_a: [512, 1536] @ [1536, D] -> [512, D]
        qf_cols = qb_cols @ q_a_w  # fp32
        kb_cols = np.concatenate([kvb[h0, :NOPE], kvb[h1, :NOPE]], axis=0)  # [256, 512]
        vb_cols = np.concatenate([kvb[h0, NOPE:], kvb[h1, NOPE:]], axis=0)  # [256, 512]
        o_slice = o_w[:, VD * h0 : VD * (h1 + 1)]  # [D, 256]
        blk = slice(SB * c, SB * (c + 1))
        per_core.append(
            {
                "qfT": np.ascontiguousarray(qf_cols.T).astype(BF16),   # [D, 512]
                "kbT": np.ascontiguousarray(kb_cols.T).astype(BF16),   # [512, 256]
                "vbT": np.ascontiguousarray(vb_cols.T).astype(BF16),   # [512, 256]
                "owT": np.ascontiguousarray(o_slice.T).astype(BF16),   # [256, D]
                "hb": np.ascontiguousarray(shared["hT"][:, blk]),      # [D, 256]
                "cosk": np.ascontiguousarray(cos128[:, blk]),          # [128, 256]
                "sink": np.ascontiguousarray(sin128[:, blk]),          # [128, 256]
            }
        )
    return shared, per_core


# ----------------------------------------------------------------------------
# numpy simulation of the device program (for host-side validation)
# ----------------------------------------------------------------------------

def _sim_phase_a(shared, pc):
    """Per-core sharded projections: returns (inv_blk [SB], kv_shard [576, SB])."""
    bf = lambda x: x.astype(BF16).astype(np.float32)
    hb = pc["hb"].astype(np.float32)              # [D, SB]
    qaT = shared["qaT"].astype(np.float32)        # [D, 1536]
    kvaT = shared["kvaT"].astype(np.float32)      # [D, 640]
    cos = pc["cosk"]                              # [128, SB]
    sin = pc["sink"]

    ckvT = kvaT.T @ hb                            # [640, SB]
    ckvb = bf(ckvT[:KV_LORA])
    ssc = (bf(ckvb * ckvb)).sum(axis=0)
    invc = 1.0 / np.sqrt(ssc / KV_LORA + EPS)
    ckvn = bf(ckvb * invc)                        # [512, SB]
    kpe, kpe2 = ckvT[512:576], ckvT[576:640]
    kper = bf(kpe * cos[0:64] + kpe2 * sin[0:64])  # [64, SB]
    kv_shard = np.concatenate([ckvn, kper], axis=0)  # [576, SB]

    qaTx = qaT.T @ hb                             # [1536, SB]
    qaTb = bf(qaTx)
    ssq = (bf(qaTb * qaTb)).sum(axis=0)
    inv = 1.0 / np.sqrt(ssq / Q_LORA + EPS)       # [SB]
    return inv.astype(np.float32), kv_shard.astype(BF16)


def _sim_core(shared, pc, inv, ckvn, kper):
    bf = lambda x: x.astype(BF16).astype(np.float32)
    hT = shared["hT"].astype(np.float32)          # [D, S]
    cos = shared["cosb"]                          # [128, S]
    sin = shared["sinb"]
    qfT = pc["qfT"].astype(np.float32)            # [D, 512]
    kbT = pc["kbT"].astype(np.float32)            # [512, 256]
    vbT = pc["vbT"].astype(np.float32)            # [512, 256]
    owT = pc["owT"].astype(np.float32)            # [256, D]

    qT = qfT.T @ hT                               # [512, S]
    qn0 = bf(qT[0:128] * inv)
    qn1 = bf(qT[128:256] * inv)
    pe, pe2 = qT[256:384], qT[384:512]
    qpe = bf((pe * cos + pe2 * sin) * inv)        # [128, S] packed (h0;h1)

    ckvn = ckvn.astype(np.float32)                # [512, S]
    kper = kper.astype(np.float32)                # [64, S]

    out = np.zeros((S, D), dtype=np.float32)
    for j in range(HPC):
        knT = bf(kbT[:, 128 * j : 128 * (j + 1)].T @ ckvn)   # [128, S]
        v = bf(ckvn.T @ vbT[:, 128 * j : 128 * (j + 1)])     # [S, 128]
        qn = qn0 if j == 0 else qn1
        qp = qpe[64 * j : 64 * (j + 1)]
        scores = knT.T @ qn + kper.T @ qp         # [S(k), S(q)] -> st[k, q]
        kidx = np.arange(S)[:, None]
        qidx = np.arange(S)[None, :]
        p = np.exp(scores) * (kidx <= qidx)
        p = bf(p)
        rs = p.sum(axis=0)                        # [q]
        oT = (v.T @ p)                            # [128, q]
        oT = bf(oT * (1.0 / rs))
        out += oT.T @ owT[128 * j : 128 * (j + 1)]
    return out


def sim(inputs):
    shared, per_core = _prep_inputs(inputs)
    invs, shards = [], []
    for c in range(N_CORES):
        inv, shard = _sim_phase_a(shared, per_core[c])
        invs.append(inv)
        shards.append(shard)
    inv = np.concatenate(invs)                    # [S]
    gathered = np.concatenate(shards, axis=1)     # [576, S]
    ckvn, kper = gathered[:KV_LORA], gathered[KV_LORA:]
    out = np.zeros((S, D), dtype=np.float32)
    for c in range(N_CORES):
        out += _sim_core(shared, per_core[c], inv, ckvn, kper)
    return out.reshape(B, S, D)


# ----------------------------------------------------------------------------
# bass program
# ----------------------------------------------------------------------------

def _split_waits(nc, max_waits=1):
    """This walrus build accepts at most one sem wait per instruction; hoist
    excess waits onto pure-wait EventSemaphore carriers just before it."""
    n_new = 0
    for f in nc.m.functions:
        for blk in f.blocks:
            new_insts = []
            for inst in blk.instructions:
                si = getattr(inst, "sync_info", None)
                waits = list(si.on_wait) if (si is not None and si.on_wait) else []
                if len(waits) > max_waits:
                    extra, keep = waits[:-max_waits], waits[-max_waits:]
                    for w in extra:
                        n_new += 1
                        carrier = mybir.InstEventSemaphore(
                            name=f"ws-{n_new}-{inst.name}",
                            engine=inst.engine,
                            ins=[],
                            outs=[],
                            sync_info=mybir.SyncInfo(on_wait=[w], on_update=[]),
                        )
                        nc.register_instruction(carrier, overwrite=True)
                        new_insts.append(carrier)
                    si.on_wait = keep
                new_insts.append(inst)
            blk.instructions = new_insts
    return n_new


def _build_nc():
    nc = bass.Bass(num_devices=N_CORES)
    hT = nc.dram_tensor("hT", [D, S], BF, kind="ExternalInput")
    hb = nc.dram_tensor("hb", [D, SB], BF, kind="ExternalInput")
    qaT = nc.dram_tensor("qaT", [D, Q_LORA], BF, kind="ExternalInput")
    kvaT = nc.dram_tensor("kvaT", [D, 640], BF, kind="ExternalInput")
    qfT = nc.dram_tensor("qfT", [D, 512], BF, kind="ExternalInput")
    kbT = nc.dram_tensor("kbT", [KV_LORA, 256], BF, kind="ExternalInput")
    vbT = nc.dram_tensor("vbT", [KV_LORA, 256], BF, kind="ExternalInput")
    owT = nc.dram_tensor("owT", [2 * VD, D], BF, kind="ExternalInput")
    cosb = nc.dram_tensor("cosb", [128, S], F32, kind="ExternalInput")
    sinb = nc.dram_tensor("sinb", [128, S], F32, kind="ExternalInput")
    cosk = nc.dram_tensor("cosk", [128, SB], F32, kind="ExternalInput")
    sink = nc.dram_tensor("sink", [128, SB], F32, kind="ExternalInput")
    maskb = nc.dram_tensor("maskb", [128, 1024], BF, kind="ExternalInput")
    out = nc.dram_tensor("out", [S, D], F32, kind="ExternalOutput")

    QL_T = Q_LORA // 128  # 12
    D_T = D // 128        # 16
    CV_T = KV_LORA // 128  # 4
    RG = [list(range(N_CORES))]

    with tile.TileContext(nc) as tc:
        with tc.tile_pool(name="persist1", bufs=1) as persist1, \
             tc.tile_pool(name="dram", bufs=1, space="DRAM") as dram:
            ones_t = persist1.tile([128, 128], BF, tag="ones")
            ones_f = persist1.tile([1, 128], F32, tag="onesf")
            eps_t = persist1.tile([128, 1], F32, tag="eps")
            nc.vector.memset(eps_t, EPS)
            nc.vector.memset(ones_t, 1.0)
            nc.vector.memset(ones_f, 1.0)
            qn_T = [persist1.tile([128, S], BF, tag=f"qnT{h}", name=f"qnT{h}") for h in range(HPC)]
            qpeP = persist1.tile([128, S], BF, tag="qpeP")
            ckvn = [persist1.tile([128, S], BF, tag=f"ckvn{i}", name=f"ckvn{i}") for i in range(CV_T)]
            kperLo = persist1.tile([128, S], BF, tag="kperLo")
            kperHi = persist1.tile([128, S], BF, tag="kperHi")
            inv_sb = persist1.tile([1, S], F32, tag="invsb")
            nc.vector.memset(kperLo[64:128, :], 0.0)
            nc.vector.memset(kperHi[0:64, :], 0.0)

            kv_in = dram.tile([576, SB], BF, tag="kvin")
            kv_out = dram.tile([576 * N_CORES, SB], BF, tag="kvout")
            inv_in = dram.tile([1, SB], F32, tag="invin")
            inv_out = dram.tile([N_CORES, SB], F32, tag="invout")

            # ------------- phase A: sharded kv_a + q_a-rms for this core's block -------------
            with tc.tile_pool(name="qaw", bufs=1) as qaw, \
                 tc.tile_pool(name="kvw", bufs=1) as kvw, \
                 tc.tile_pool(name="hbx", bufs=1) as hbx, \
                 tc.tile_pool(name="cskp", bufs=1) as cskp, \
                 tc.tile_pool(name="cvsb", bufs=1) as cvsb, \
                 tc.tile_pool(name="sq", bufs=2) as sqp, \
                 tc.tile_pool(name="nrm", bufs=2) as nrm, \
                 tc.tile_pool(name="pet", bufs=1) as pet, \
                 tc.tile_pool(name="st_ps", bufs=3, space="PSUM") as st_ps, \
                 tc.tile_pool(name="ssq_ps", bufs=1, space="PSUM") as ssq_ps, \
                 tc.tile_pool(name="ssq2_ps", bufs=1, space="PSUM") as ssq2_ps:

                kva_w = kvw.tile([128, D_T, 640], BF, tag="kvw")
                hb_t = hbx.tile([128, D_T, SB], BF, tag="hb")
                for k in range(D_T):
                    nc.sync.dma_start(out=kva_w[:, k, :], in_=kvaT[128 * k : 128 * (k + 1), :])
                    nc.sync.dma_start(out=hb_t[:, k, :], in_=hb[128 * k : 128 * (k + 1), :])
                cos_c = cskp.tile([128, SB], F32, tag="coskc")
                sin_c = cskp.tile([128, SB], F32, tag="sinkc")
                nc.sync.dma_start(out=cos_c, in_=cosk[:, :])
                nc.sync.dma_start(out=sin_c, in_=sink[:, :])

                # ---- kv_a: 4 ckv m-tiles + kpe + kpe2 ----
                cv_t = cvsb.tile([128, CV_T, SB], BF, tag="cv")
                cvn_t = cvsb.tile([128, CV_T, SB], BF, tag="cvn")
                ssc = ssq2_ps.tile([128, SB], F32, tag="ssc")
                pe_ps = []
                for m in range(6):
                    mp = 128 if m < 4 else 64
                    col = slice(128 * m, 128 * m + 128) if m < 4 else \
                        slice(512 + 64 * (m - 4), 512 + 64 * (m - 3))
                    ps = st_ps.tile([mp, SB], F32, tag="stps")
                    for k in range(D_T):
                        nc.tensor.matmul(
                            ps,
                            kva_w[:, k, col],
                            hb_t[:, k, :],
                            start=(k == 0),
                            stop=(k == D_T - 1),
                        )
                    if m < 4:
                        nc.vector.tensor_copy(cv_t[:, m, :], ps)
                        sq = sqp.tile([128, SB], BF, tag="sq")
                        nc.scalar.activation(out=sq, in_=ps, func=AF.Square)
                        nc.tensor.matmul(
                            ssc, ones_t, sq, start=(m == 0), stop=(m == CV_T - 1)
                        )
                    else:
                        pe_ps.append(ps)

                # ---- kv norm + kpe rope; ship shard ----
                bc2 = nrm.tile([128, SB], F32, tag="bc2")
                nc.scalar.activation(
                    out=bc2, in_=ssc, func=AF.Sqrt, scale=1.0 / KV_LORA, bias=eps_t
                )
                nc.vector.reciprocal(bc2, bc2)
                for i in range(CV_T):
                    nc.vector.tensor_mul(cvn_t[:, i, :], cv_t[:, i, :], bc2)
                    nc.gpsimd.dma_start(kv_in[128 * i : 128 * (i + 1), :], cvn_t[:, i, :])
                t1 = pet.tile([128, SB], F32, tag="t1")
                t2 = pet.tile([128, SB], F32, tag="t2")
                kper_sh = pet.tile([64, SB], BF, tag="kpersh")
                nc.vector.tensor_mul(t1[0:64, :], pe_ps[0], cos_c[0:64, :])
                nc.vector.tensor_mul(t2[0:64, :], pe_ps[1], sin_c[0:64, :])
                nc.vector.tensor_add(kper_sh[:, :], t1[0:64, :], t2[0:64, :])
                nc.gpsimd.dma_start(kv_in[512:576, :], kper_sh[:, :])
                nc.gpsimd.collective_compute(
                    "AllGather",
                    mybir.AluOpType.bypass,
                    replica_groups=RG,
                    ins=[kv_in[:]],
                    outs=[kv_out[:]],
                )

                # ---- q_a: 12 m-tiles, squares only (rms stat) ----
                qa_w = qaw.tile([128, D_T, Q_LORA], BF, tag="qaw")
                for k in range(D_T):
                    nc.sync.dma_start(out=qa_w[:, k, :], in_=qaT[128 * k : 128 * (k + 1), :])
                ssq = ssq_ps.tile([128, SB], F32, tag="ssq")
                for m in range(QL_T):
                    ps = st_ps.tile([128, SB], F32, tag="stps")
                    for k in range(D_T):
                        nc.tensor.matmul(
                            ps,
                            qa_w[:, k, 128 * m : 128 * (m + 1)],
                            hb_t[:, k, :],
                            start=(k == 0),
                            stop=(k == D_T - 1),
                        )
                    sq = sqp.tile([128, SB], BF, tag="sq")
                    nc.scalar.activation(out=sq, in_=ps, func=AF.Square)
                    nc.tensor.matmul(
                        ssq, ones_t, sq, start=(m == 0), stop=(m == QL_T - 1)
                    )
                bcq = nrm.tile([128, SB], F32, tag="bcq")
                nc.scalar.activation(
                    out=bcq, in_=ssq, func=AF.Sqrt, scale=1.0 / Q_LORA, bias=eps_t
                )
                nc.vector.reciprocal(bcq, bcq)
                nc.gpsimd.dma_start(inv_in[0:1, :], bcq[0:1, :])
                nc.gpsimd.collective_compute(
                    "AllGather",
                    mybir.AluOpType.bypass,
                    replica_groups=RG,
                    ins=[inv_in[:]],
                    outs=[inv_out[:]],
                )

            # ------------- phase B: fused q projection over all chunks -------------
            with tc.tile_pool(name="qfw", bufs=1) as qfw, \
                 tc.tile_pool(name="hx", bufs=2) as hx, \
                 tc.tile_pool(name="csp", bufs=2) as csp, \
                 tc.tile_pool(name="bcp", bufs=2) as bcp, \
                 tc.tile_pool(name="pet2", bufs=1) as pet2, \
                 tc.tile_pool(name="qt_ps", bufs=4, space="PSUM") as qt_ps, \
                 tc.tile_pool(name="bc_ps", bufs=2, space="PSUM") as bc_ps:

                qf_w = qfw.tile([128, D_T, 512], BF, tag="qfw")
                for k in range(D_T):
                    nc.sync.dma_start(out=qf_w[:, k, :], in_=qfT[128 * k : 128 * (k + 1), :])
                for b in range(N_CORES):
                    nc.sync.dma_start(
                        out=inv_sb[0:1, SB * b : SB * (b + 1)], in_=inv_out[b : b + 1, :]
                    )

                for c in range(NCHUNK):
                    cs = slice(NQ * c, NQ * (c + 1))
                    h_t = hx.tile([128, D_T, NQ], BF, tag="h")
                    for k in range(D_T):
                        nc.sync.dma_start(out=h_t[:, k, :], in_=hT[128 * k : 128 * (k + 1), cs])
                    cos_c = csp.tile([128, NQ], F32, tag="cosc")
                    sin_c = csp.tile([128, NQ], F32, tag="sinc")
                    nc.sync.dma_start(out=cos_c, in_=cosb[:, cs])
                    nc.sync.dma_start(out=sin_c, in_=sinb[:, cs])

                    bc_p = bc_ps.tile([128, NQ], F32, tag="bcps")
                    nc.tensor.matmul(
                        bc_p, ones_f, inv_sb[0:1, cs], start=True, stop=True
                    )
                    bc = bcp.tile([128, NQ], F32, tag="bc")
                    nc.scalar.activation(out=bc, in_=bc_p, func=AF.Copy)

                    qt_tiles = []
                    for b in range(4):
                        ps = qt_ps.tile([128, NQ], F32, tag="qtps")
                        for k in range(D_T):
                            nc.tensor.matmul(
                                ps,
                                qf_w[:, k, 128 * b : 128 * (b + 1)],
                                h_t[:, k, :],
                                start=(k == 0),
                                stop=(k == D_T - 1),
                            )
                        if b == 0:
                            nc.vector.tensor_mul(qn_T[0][:, cs], ps, bc)
                        elif b == 1:
                            nc.vector.tensor_mul(qn_T[1][:, cs], ps, bc)
                        else:
                            qt_tiles.append(ps)
                    t1 = pet2.tile([128, NQ], F32, tag="t1")
                    t2 = pet2.tile([128, NQ], F32, tag="t2")
                    nc.vector.tensor_mul(t1, qt_tiles[0], cos_c)
                    nc.vector.tensor_mul(t2, qt_tiles[1], sin_c)
                    nc.vector.tensor_add(t1, t1, t2)
                    nc.vector.tensor_mul(qpeP[:, cs], t1, bc)

            # ---------------- phase B2: unpack gathered kv + kv_b projections ----------------
            with tc.tile_pool(name="persist2", bufs=1) as persist2:
                kn_T = [persist2.tile([128, S], BF, tag=f"knT{h}", name=f"knT{h}") for h in range(HPC)]
                v_sb = [persist2.tile([128, S], BF, tag=f"v{h}", name=f"v{h}") for h in range(HPC)]
                o_T = [persist2.tile([128, S], BF, tag=f"oT{h}", name=f"oT{h}") for h in range(HPC)]
                with tc.tile_pool(name="kbw", bufs=1) as kbw, \
                     tc.tile_pool(name="kn_ps", bufs=2, space="PSUM") as kn_ps, \
                     tc.tile_pool(name="v_ps", bufs=3, space="PSUM") as v_ps:
                    for b in range(N_CORES):
                        bs = slice(SB * b, SB * (b + 1))
                        for ct in range(CV_T):
                            nc.sync.dma_start(
                                out=ckvn[ct][:, bs],
                                in_=kv_out[576 * b + 128 * ct : 576 * b + 128 * (ct + 1), :],
                            )
                        nc.sync.dma_start(
                            out=kperLo[0:64, bs], in_=kv_out[576 * b + 512 : 576 * b + 576, :]
                        )
                        nc.sync.dma_start(
                            out=kperHi[64:128, bs], in_=kv_out[576 * b + 512 : 576 * b + 576, :]
                        )
                    kb_w = kbw.tile([128, CV_T, 256], BF, tag="kbw")
                    vb_w = kbw.tile([128, CV_T, 256], BF, tag="vbw")
                    for ct in range(CV_T):
                        nc.sync.dma_start(out=kb_w[:, ct, :], in_=kbT[128 * ct : 128 * (ct + 1), :])
                        nc.sync.dma_start(out=vb_w[:, ct, :], in_=vbT[128 * ct : 128 * (ct + 1), :])
                    for h in range(HPC):
                        hs = slice(128 * h, 128 * (h + 1))
                        for c in range(NCHUNK):
                            cs = slice(NQ * c, NQ * (c + 1))
                            ps = kn_ps.tile([128, NQ], F32, tag="knps")
                            for ct in range(CV_T):
                                nc.tensor.matmul(
                                    ps,
                                    kb_w[:, ct, hs],
                                    ckvn[ct][:, cs],
                                    start=(ct == 0),
                                    stop=(ct == CV_T - 1),
                                )
                            nc.vector.tensor_copy(kn_T[h][:, cs], ps)
                        for kt in range(KT):
                            ks = slice(128 * kt, 128 * (kt + 1))
                            ps = v_ps.tile([128, VD], F32, tag="vps")
                            for ct in range(CV_T):
                                nc.tensor.matmul(
                                    ps,
                                    ckvn[ct][:, ks],
                                    vb_w[:, ct, hs],
                                    start=(ct == 0),
                                    stop=(ct == CV_T - 1),
                                )
                            nc.vector.tensor_copy(v_sb[h][:, ks], ps)

                # ---------------- phase C: attention ----------------
                mskp_cm = tc.tile_pool(name="mskp", bufs=1)
                oww_cm = tc.tile_pool(name="oww", bufs=1)
                mskp = mskp_cm.__enter__()
                oww = oww_cm.__enter__()
                with tc.tile_pool(name="pp", bufs=6) as pp, \
                     tc.tile_pool(name="ep", bufs=3) as ep, \
                     tc.tile_pool(name="rvp", bufs=2) as rvp, \
                     tc.tile_pool(name="ostg", bufs=4) as ostg, \
                     tc.tile_pool(name="s_ps", bufs=3, space="PSUM") as s_ps, \
                     tc.tile_pool(name="rs_ps", bufs=2, space="PSUM") as rs_ps, \
                     tc.tile_pool(name="o_ps", bufs=2, space="PSUM") as o_ps, \
                     tc.tile_pool(name="out_ps", bufs=1, space="PSUM") as out_ps:
                    mask_s = mskp.tile([128, 1024], BF, tag="mask")
                    nc.sync.dma_start(out=mask_s, in_=maskb[:, :])
                    ow_t = oww.tile([128, HPC, D], BF, tag="oww")
                    for j in range(HPC):
                        nc.sync.dma_start(out=ow_t[:, j, :], in_=owT[128 * j : 128 * (j + 1), :])
                    for c in range(NCHUNK):
                        cs = slice(NQ * c, NQ * (c + 1))
                        nkt = 4 * (c + 1)
                        for h in range(HPC):
                            kper_h = kperLo if h == 0 else kperHi
                            rs = rs_ps.tile([128, NQ], F32, tag="rs")
                            op = o_ps.tile([128, NQ], F32, tag="op")
                            for kt in range(nkt):
                                ks = slice(128 * kt, 128 * (kt + 1))
                                i = kt - 4 * c
                                lo = 128 * i if i > 0 else 0  # valid q-subrange start
                                qs = slice(NQ * c + lo, NQ * (c + 1))
                                vs = slice(lo, NQ)
                                sp = s_ps.tile([128, NQ], F32, tag="sp")
                                nc.tensor.matmul(
                                    sp[:, vs], kn_T[h][:, ks], qn_T[h][:, qs],
                                    start=True, stop=False,
                                )
                                nc.tensor.matmul(
                                    sp[:, vs], kper_h[:, ks], qpeP[:, qs],
                                    start=False, stop=True,
                                )
                                p_t = pp.tile([128, NQ], BF, tag="p")
                                if kt >= 4 * c:
                                    e_t = ep.tile([128, NQ], BF, tag="e")
                                    nc.scalar.activation(out=e_t[:, vs], in_=sp[:, vs], func=AF.Exp)
                                    nc.vector.tensor_mul(
                                        p_t[:, vs], e_t[:, vs],
                                        mask_s[:, 384 : 896 - lo],
                                    )
                                else:
                                    nc.scalar.activation(out=p_t[:, vs], in_=sp[:, vs], func=AF.Exp)
                                nc.tensor.matmul(
                                    rs[:, vs], ones_t, p_t[:, vs],
                                    start=(kt == 0), stop=(kt == nkt - 1),
                                )
                                nc.tensor.matmul(
                                    op[:, vs],
                                    v_sb[h][:, ks],
                                    p_t[:, vs],
                                    start=(kt == 0), stop=(kt == nkt - 1),
                                )
                            rv = rvp.tile([128, NQ], F32, tag="rv")
                            nc.vector.reciprocal(rv, rs)
                            nc.vector.tensor_mul(o_T[h][:, cs], op, rv)
                        # o_proj for this chunk's 4 s-tiles (both heads now done;
                        # last chunk handled in a post-phase with deeper PSUM)
                        for si in range(4 * c, 4 * (c + 1) if c < NCHUNK - 1 else 4 * c):
                            ss = slice(128 * si, 128 * (si + 1))
                            for nch in range(NCHUNK):
                                ns = slice(NQ * nch, NQ * (nch + 1))
                                ps = out_ps.tile([128, NQ], F32, tag="outps")
                                for j in range(HPC):
                                    nc.tensor.matmul(
                                        ps,
                                        o_T[j][:, ss],
                                        ow_t[:, j, ns],
                                        start=(j == 0),
                                        stop=(j == HPC - 1),
                                    )
                                stg = ostg.tile([128, NQ], F32, tag="ostg")
                                nc.scalar.activation(out=stg, in_=ps, func=AF.Copy)
                                nc.sync.dma_start(out=out[ss, ns], in_=stg)
                # ---------------- final chunk o_proj ----------------
                with tc.tile_pool(name="ostg2", bufs=4) as ostg2, \
                     tc.tile_pool(name="out2_ps", bufs=4, space="PSUM") as out2_ps:
                    for si in range(4 * (NCHUNK - 1), 4 * NCHUNK):
                        ss = slice(128 * si, 128 * (si + 1))
                        for nch in range(NCHUNK):
                            ns = slice(NQ * nch, NQ * (nch + 1))
                            ps = out2_ps.tile([128, NQ], F32, tag="out2ps")
                            for j in range(HPC):
                                nc.tensor.matmul(
                                    ps,
                                    o_T[j][:, ss],
                                    ow_t[:, j, ns],
                                    start=(j == 0),
                                    stop=(j == HPC - 1),
                                )
                            stg = ostg2.tile([128, NQ], F32, tag="ostg2")
                            nc.scalar.activation(out=stg, in_=ps, func=AF.Copy)
                            nc.sync.dma_start(out=out[ss, ns], in_=stg)
                oww_cm.__exit__(None, None, None)
                mskp_cm.__exit__(None, None, None)

    _split_waits(nc)
    return nc


# ----------------------------------------------------------------------------
# entry point
# ----------------------------------------------------------------------------

def kernel(**inputs):
    global LAST_RESULTS
    shared, per_core = _prep_inputs(inputs)
    if "nc" not in _CACHE:
        _CACHE["nc"] = _build_nc()
    nc = _CACHE["nc"]
    in_maps = []
    for c in range(N_CORES):
        m = {
            "hT": shared["hT"],
            "qaT": shared["qaT"],
            "kvaT": shared["kvaT"],
            "cosb": shared["cosb"],
            "sinb": shared["sinb"],
            "maskb": shared["maskb"],
            "qfT": per_core[c]["qfT"],
            "kbT": per_core[c]["kbT"],
            "vbT": per_core[c]["vbT"],
            "owT": per_core[c]["owT"],
            "hb": per_core[c]["hb"],
            "cosk": per_core[c]["cosk"],
            "sink": per_core[c]["sink"],
        }
        in_maps.append(m)
    res = run_bass_kernel_spmd(nc, in_maps, core_ids=list(range(N_CORES)))
    LAST_RESULTS = res
    out = np.zeros((S, D), dtype=np.float32)
    for r in res.results:
        out += r["out"]
    return out.reshape(B, S, D)


# revision 6
# speedup vs baseline: 1.3943x; 1.0485x over previous
"""DeepseekV3 MLA attention (B=1, S=2048, D=2048, H=16) on 8 trn2 NeuronCores.

Strategy (v3: tensor-parallel heads + sharded shared projections + on-device
AllGather, ordered so both collectives hide behind the fused q projection):
  - the q_b@q_a product is FUSED on the host per core's 2 heads, so each core
    computes q directly from hidden states (contraction D=2048) and nobody
    materializes the full q_a activation;
  - the q rmsnorm statistic is sharded: core c computes q_a only for seq
    positions [256c, 256c+256) (seq on PSUM partitions so the square-sum
    runs on the vector engine), reduces to a 1/rms row, and an 8-core
    AllGather ([1,256] f32 per core) distributes it;
  - kv_a is sharded the same way: core c computes ckv for its 256 positions,
    normalizes + ropes locally, and an AllGather ([576,256] bf16 per core)
    distributes the shared K/V latents to every core;
  - each core owns 2 heads: kv_b, causal flash-style attention (no max
    subtraction -- logits are O(1) here), and its slice of o_proj, producing
    a partial [S, D] output; host sums the 8 partials.

All matmuls run in bf16 (fp32 PSUM accumulation); rmsnorm stats, rope and
softmax run in fp32.  RoPE deinterleave + rotate-half are folded into the
weight layout on the host (extra "pre-swapped, sign-folded" weight columns)
so the device only does aligned elementwise mul/adds.
"""

import numpy as np
import ml_dtypes

import concourse.bass as bass
import concourse.mybir as mybir
import concourse.tile as tile
from concourse.bass_utils import run_bass_kernel_spmd

BF16 = ml_dtypes.bfloat16
F32 = mybir.dt.float32
BF = mybir.dt.bfloat16

B, S, D = 1, 2048, 2048
H = 16
N_CORES = 8
HPC = H // N_CORES  # heads per core = 2
Q_LORA = 1536
KV_LORA = 512
NOPE = 128
ROPE = 64
VD = 128
QHD = NOPE + ROPE  # 192
THETA = 50000.0
EPS = 1e-6
SCALE = QHD ** (-0.5)

NQ = 512            # q-chunk (matmul free dim)
NCHUNK = S // NQ    # 4
KT = S // 128       # 16 k-tiles
SB = S // N_CORES   # 256: per-core seq block for the sharded projections
AF = mybir.ActivationFunctionType
ALU = mybir.AluOpType

LAST_RESULTS = None
_CACHE = {}


# ----------------------------------------------------------------------------
# host-side weight preparation
# ----------------------------------------------------------------------------

def _deint_perm():
    # deinterleave: out[j] = in[2j] (j<32), in[2(j-32)+1] (j>=32)
    p = np.empty(ROPE, dtype=np.int64)
    p[:32] = 2 * np.arange(32)
    p[32:] = 2 * np.arange(32) + 1
    return p


def _rope_tables(position_ids):
    pos = np.asarray(position_ids).reshape(-1).astype(np.float32)  # [S]
    inv_freq = (1.0 / (THETA ** (np.arange(0, ROPE, 2, dtype=np.float32) / ROPE)))
    freqs = np.outer(pos, inv_freq)  # [S, 32]
    cos32 = np.cos(freqs).T.astype(np.float32)  # [32, S]
    sin32 = np.sin(freqs).T.astype(np.float32)
    cos128 = np.tile(cos32, (4, 1))  # [128, S]
    sin128 = np.tile(sin32, (4, 1))
    return cos128, sin128


def _causal_mask_big():
    # M[dk, u] = 1 if u >= dk + 384 ; slice [:, 384-128*i : 896-128*i]
    # gives the diagonal-block mask indicator(dq >= dk + 128*i)
    dk = np.arange(128)[:, None]
    u = np.arange(1024)[None, :]
    return (u >= dk + 384).astype(BF16)


def _prep_inputs(inputs):
    hidden = np.asarray(inputs["hidden_states"], dtype=np.float32)[0]  # [S, D]
    position_ids = np.asarray(inputs["position_ids"])
    q_a_w = np.asarray(inputs["q_a_w"], dtype=np.float32)        # [1536, D]
    q_a_ln_w = np.asarray(inputs["q_a_ln_w"], dtype=np.float32)  # [1536]
    q_b_w = np.asarray(inputs["q_b_w"], dtype=np.float32)        # [H*192, 1536]
    kv_a_w = np.asarray(inputs["kv_a_w"], dtype=np.float32)      # [576, D]
    kv_a_ln_w = np.asarray(inputs["kv_a_ln_w"], dtype=np.float32)  # [512]
    kv_b_w = np.asarray(inputs["kv_b_w"], dtype=np.float32)      # [H*256, 512]
    o_w = np.asarray(inputs["o_w"], dtype=np.float32)            # [D, H*128]

    dp = _deint_perm()
    dps = dp[(np.arange(ROPE) ^ 32)]          # source index for the swapped term
    sgn = np.where(np.arange(ROPE) < 32, -1.0, 1.0).astype(np.float32)[:, None]

    shared = {}
    shared["hT"] = np.ascontiguousarray(hidden.T).astype(BF16)          # [D, S]
    shared["qaT"] = np.ascontiguousarray(q_a_w.T).astype(BF16)          # [D, 1536]

    # kv_a columns: [ckv 512 | kpe 64 (deint) | kpe2 64 (swap+sign)]
    kva_cols = np.concatenate(
        [kv_a_w[:KV_LORA], kv_a_w[KV_LORA + dp], sgn * kv_a_w[KV_LORA + dps]], axis=0
    )  # [640, D]
    shared["kvaT"] = np.ascontiguousarray(kva_cols.T).astype(BF16)      # [D, 640]

    cos128, sin128 = _rope_tables(position_ids)
    shared["cosb"] = cos128
    shared["sinb"] = sin128
    shared["maskb"] = _causal_mask_big()

    # q_b with ln + scale folded
    qb = q_b_w * q_a_ln_w[None, :] * SCALE  # [H*192, 1536]
    qb = qb.reshape(H, QHD, Q_LORA)
    kvb = (kv_b_w * kv_a_ln_w[None, :]).reshape(H, NOPE + VD, KV_LORA)

    per_core = []
    for c in range(N_CORES):
        h0, h1 = HPC * c, HPC * c + 1
        nope0 = qb[h0, :NOPE]            # [128, 1536]
        nope1 = qb[h1, :NOPE]
        peP = np.concatenate([qb[h0, NOPE + dp], qb[h1, NOPE + dp]], axis=0)  # [128,...]
        pe2P = np.concatenate(
            [sgn * qb[h0, NOPE + dps], sgn * qb[h1, NOPE + dps]], axis=0
        )
        qb_cols = np.concatenate([nope0, nope1, peP, pe2P], axis=0)  # [512, 1536]
        # fuse q_b @ q_a: [512, 1536] @ [1536, D] -> [512, D]
        qf_cols = qb_cols @ q_a_w  # fp32
        kb_cols = np.concatenate([kvb[h0, :NOPE], kvb[h1, :NOPE]], axis=0)  # [256, 512]
        vb_cols = np.concatenate([kvb[h0, NOPE:], kvb[h1, NOPE:]], axis=0)  # [256, 512]
        o_slice = o_w[:, VD * h0 : VD * (h1 + 1)]  # [D, 256]
        blk = slice(SB * c, SB * (c + 1))
        per_core.append(
            {
                "qfT": np.ascontiguousarray(qf_cols.T).astype(BF16),   # [D, 512]
                "kbT": np.ascontiguousarray(kb_cols.T).astype(BF16),   # [512, 256]
                "vbT": np.ascontiguousarray(vb_cols.T).astype(BF16),   # [512, 256]
                "owT": np.ascontiguousarray(o_slice.T).astype(BF16),   # [256, D]
                "hb": np.ascontiguousarray(shared["hT"][:, blk]),      # [D, 256]
                "cosk": np.ascontiguousarray(cos128[:, blk]),          # [128, 256]
                "sink": np.ascontiguousarray(sin128[:, blk]),          # [128, 256]
            }
        )
    return shared, per_core


# ----------------------------------------------------------------------------
# numpy simulation of the device program (for host-side validation)
# ----------------------------------------------------------------------------

def _sim_phase_a(shared, pc):
    """Per-core sharded projections: returns (inv_blk [SB], kv_shard [576, SB])."""
    bf = lambda x: x.astype(BF16).astype(np.float32)
    hb = pc["hb"].astype(np.float32)              # [D, SB]
    qaT = shared["qaT"].astype(np.float32)        # [D, 1536]
    kvaT = shared["kvaT"].astype(np.float32)      # [D, 640]
    cos = pc["cosk"]                              # [128, SB]
    sin = pc["sink"]

    ckvT = kvaT.T @ hb                            # [640, SB]
    ckvb = bf(ckvT[:KV_LORA])
    ssc = (bf(ckvb * ckvb)).sum(axis=0)
    invc = 1.0 / np.sqrt(ssc / KV_LORA + EPS)
    ckvn = bf(ckvb * invc)                        # [512, SB]
    kpe, kpe2 = ckvT[512:576], ckvT[576:640]
    kper = bf(kpe * cos[0:64] + kpe2 * sin[0:64])  # [64, SB]
    kv_shard = np.concatenate([ckvn, kper], axis=0)  # [576, SB]

    qaTx = qaT.T @ hb                             # [1536, SB] (fp32 PSUM values)
    ssq = bf(qaTx * qaTx).sum(axis=0)             # bf16 squares, fp32 reduce
    inv = 1.0 / np.sqrt(ssq / Q_LORA + EPS)       # [SB]
    return inv.astype(np.float32), kv_shard.astype(BF16)


def _sim_core(shared, pc, inv, ckvn, kper):
    bf = lambda x: x.astype(BF16).astype(np.float32)
    hT = shared["hT"].astype(np.float32)          # [D, S]
    cos = shared["cosb"]                          # [128, S]
    sin = shared["sinb"]
    qfT = pc["qfT"].astype(np.float32)            # [D, 512]
    kbT = pc["kbT"].astype(np.float32)            # [512, 256]
    vbT = pc["vbT"].astype(np.float32)            # [512, 256]
    owT = pc["owT"].astype(np.float32)            # [256, D]

    qT = qfT.T @ hT                               # [512, S]
    qn0 = bf(qT[0:128] * inv)
    qn1 = bf(qT[128:256] * inv)
    pe, pe2 = qT[256:384], qT[384:512]
    qpe = bf((pe * cos + pe2 * sin) * inv)        # [128, S] packed (h0;h1)

    ckvn = ckvn.astype(np.float32)                # [512, S]
    kper = kper.astype(np.float32)                # [64, S]

    out = np.zeros((S, D), dtype=np.float32)
    for j in range(HPC):
        knT = bf(kbT[:, 128 * j : 128 * (j + 1)].T @ ckvn)   # [128, S]
        v = bf(ckvn.T @ vbT[:, 128 * j : 128 * (j + 1)])     # [S, 128]
        qn = qn0 if j == 0 else qn1
        qp = qpe[64 * j : 64 * (j + 1)]
        scores = knT.T @ qn + kper.T @ qp         # [S(k), S(q)] -> st[k, q]
        kidx = np.arange(S)[:, None]
        qidx = np.arange(S)[None, :]
        p = np.exp(scores) * (kidx <= qidx)
        p = bf(p)
        rs = p.sum(axis=0)                        # [q]
        oT = (v.T @ p)                            # [128, q]
        oT = bf(oT * (1.0 / rs))
        out += oT.T @ owT[128 * j : 128 * (j + 1)]
    return out


def sim(inputs):
    shared, per_core = _prep_inputs(inputs)
    invs, shards = [], []
    for c in range(N_CORES):
        inv, shard = _sim_phase_a(shared, per_core[c])
        invs.append(inv)
        shards.append(shard)
    inv = np.concatenate(invs)                    # [S]
    gathered = np.concatenate(shards, axis=1)     # [576, S]
    ckvn, kper = gathered[:KV_LORA], gathered[KV_LORA:]
    out = np.zeros((S, D), dtype=np.float32)
    for c in range(N_CORES):
        out += _sim_core(shared, per_core[c], inv, ckvn, kper)
    return out.reshape(B, S, D)


# ----------------------------------------------------------------------------
# bass program
# ----------------------------------------------------------------------------

def _split_waits(nc, max_waits=1):
    """This walrus build accepts at most one sem wait per instruction; hoist
    excess waits onto pure-wait EventSemaphore carriers just before it."""
    n_new = 0
    for f in nc.m.functions:
        for blk in f.blocks:
            new_insts = []
            for inst in blk.instructions:
                si = getattr(inst, "sync_info", None)
                waits = list(si.on_wait) if (si is not None and si.on_wait) else []
                if len(waits) > max_waits:
                    extra, keep = waits[:-max_waits], waits[-max_waits:]
                    for w in extra:
                        n_new += 1
                        carrier = mybir.InstEventSemaphore(
                            name=f"ws-{n_new}-{inst.name}",
                            engine=inst.engine,
                            ins=[],
                            outs=[],
                            sync_info=mybir.SyncInfo(on_wait=[w], on_update=[]),
                        )
                        nc.register_instruction(carrier, overwrite=True)
                        new_insts.append(carrier)
                    si.on_wait = keep
                new_insts.append(inst)
            blk.instructions = new_insts
    return n_new


def _build_nc():
    nc = bass.Bass(num_devices=N_CORES)
    hT = nc.dram_tensor("hT", [D, S], BF, kind="ExternalInput")
    hb = nc.dram_tensor("hb", [D, SB], BF, kind="ExternalInput")
    qaT = nc.dram_tensor("qaT", [D, Q_LORA], BF, kind="ExternalInput")
    kvaT = nc.dram_tensor("kvaT", [D, 640], BF, kind="ExternalInput")
    qfT = nc.dram_tensor("qfT", [D, 512], BF, kind="ExternalInput")
    kbT = nc.dram_tensor("kbT", [KV_LORA, 256], BF, kind="ExternalInput")
    vbT = nc.dram_tensor("vbT", [KV_LORA, 256], BF, kind="ExternalInput")
    owT = nc.dram_tensor("owT", [2 * VD, D], BF, kind="ExternalInput")
    cosb = nc.dram_tensor("cosb", [128, S], F32, kind="ExternalInput")
    sinb = nc.dram_tensor("sinb", [128, S], F32, kind="ExternalInput")
    cosk = nc.dram_tensor("cosk", [128, SB], F32, kind="ExternalInput")
    sink = nc.dram_tensor("sink", [128, SB], F32, kind="ExternalInput")
    maskb = nc.dram_tensor("maskb", [128, 1024], BF, kind="ExternalInput")
    out = nc.dram_tensor("out", [S, D], F32, kind="ExternalOutput")
    # collective outputs live in the Shared scratchpad (faster HBM-HBM path)
    kv_out = nc.dram_tensor("kv_out_sh", [576 * N_CORES, SB], BF, addr_space="Shared")
    inv_out = nc.dram_tensor("inv_out_sh", [N_CORES, SB], F32, addr_space="Shared")

    QL_T = Q_LORA // 128  # 12
    D_T = D // 128        # 16
    CV_T = KV_LORA // 128  # 4
    RG = [list(range(N_CORES))]

    with tile.TileContext(nc) as tc:
        with tc.tile_pool(name="persist1", bufs=1) as persist1, \
             tc.tile_pool(name="dram", bufs=1, space="DRAM") as dram:
            ones_t = persist1.tile([128, 128], BF, tag="ones")
            ones_f = persist1.tile([1, 128], F32, tag="onesf")
            eps_t = persist1.tile([128, 1], F32, tag="eps")
            nc.vector.memset(eps_t, EPS)
            nc.vector.memset(ones_t, 1.0)
            nc.vector.memset(ones_f, 1.0)
            qn_T = [persist1.tile([128, S], BF, tag=f"qnT{h}", name=f"qnT{h}") for h in range(HPC)]
            qpeP = persist1.tile([128, S], BF, tag="qpeP")
            ckvn = [persist1.tile([128, S], BF, tag=f"ckvn{i}", name=f"ckvn{i}") for i in range(CV_T)]
            kperLo = persist1.tile([128, S], BF, tag="kperLo")
            kperHi = persist1.tile([128, S], BF, tag="kperHi")
            inv_sb = persist1.tile([1, S], F32, tag="invsb")
            nc.vector.memset(kperLo[64:128, :], 0.0)
            nc.vector.memset(kperHi[0:64, :], 0.0)

            kv_in = dram.tile([576, SB], BF, tag="kvin")
            inv_in = dram.tile([1, SB], F32, tag="invin")

            # ------------- phase A: sharded q_a rms stat + kv_a for this core's block -------------
            with tc.tile_pool(name="qaw", bufs=1) as qaw, \
                 tc.tile_pool(name="kvw", bufs=1) as kvw, \
                 tc.tile_pool(name="hbx", bufs=1) as hbx, \
                 tc.tile_pool(name="cskp", bufs=1) as cskp, \
                 tc.tile_pool(name="cvsb", bufs=1) as cvsb, \
                 tc.tile_pool(name="sq", bufs=2) as sqp, \
                 tc.tile_pool(name="acc", bufs=8) as accp, \
                 tc.tile_pool(name="nrm", bufs=2) as nrm, \
                 tc.tile_pool(name="pet", bufs=1) as pet, \
                 tc.tile_pool(name="qa_ps", bufs=3, space="PSUM") as qa_ps, \
                 tc.tile_pool(name="st_ps", bufs=3, space="PSUM") as st_ps, \
                 tc.tile_pool(name="ssq2_ps", bufs=1, space="PSUM") as ssq2_ps:

                hb_t = hbx.tile([128, D_T, SB], BF, tag="hb")
                qa_w = qaw.tile([128, D_T, Q_LORA], BF, tag="qaw")
                kva_w = kvw.tile([128, D_T, 640], BF, tag="kvw")
                for k in range(D_T):
                    nc.sync.dma_start(out=hb_t[:, k, :], in_=hb[128 * k : 128 * (k + 1), :])
                for k in range(D_T):
                    eng = nc.sync if k % 2 == 0 else nc.scalar
                    eng.dma_start(out=qa_w[:, k, :], in_=qaT[128 * k : 128 * (k + 1), :])
                    nc.scalar.dma_start(out=kva_w[:, k, :], in_=kvaT[128 * k : 128 * (k + 1), :])
                cos_c = cskp.tile([128, SB], F32, tag="coskc")
                sin_c = cskp.tile([128, SB], F32, tag="sinkc")
                nc.sync.dma_start(out=cos_c, in_=cosk[:, :])
                nc.sync.dma_start(out=sin_c, in_=sink[:, :])

                # ---- A1: q_a squares, seq on partitions; DVE row-reduce ----
                for s in range(2):
                    accs = []
                    for mb in range(3):
                        ps = qa_ps.tile([128, 512], F32, tag="qaps")
                        for k in range(D_T):
                            nc.tensor.matmul(
                                ps,
                                hb_t[:, k, 128 * s : 128 * (s + 1)],
                                qa_w[:, k, 512 * mb : 512 * (mb + 1)],
                                start=(k == 0),
                                stop=(k == D_T - 1),
                            )
                        sqd = sqp.tile([128, 512], BF, tag="sqd")
                        nc.scalar.activation(out=sqd, in_=ps, func=AF.Square)
                        acc = accp.tile([128, 1], F32, tag=f"acc{s}{mb}")
                        nc.vector.reduce_sum(
                            out=acc, in_=sqd, axis=mybir.AxisListType.X
                        )
                        accs.append(acc)
                    nc.vector.tensor_add(accs[0], accs[0], accs[1])
                    nc.vector.tensor_add(accs[0], accs[0], accs[2])
                    inv_col = nrm.tile([128, 1], F32, tag=f"invc{s}")
                    nc.scalar.activation(
                        out=inv_col, in_=accs[0], func=AF.Sqrt,
                        scale=1.0 / Q_LORA, bias=eps_t,
                    )
                    nc.vector.reciprocal(inv_col, inv_col)
                    nc.gpsimd.dma_start(
                        inv_in[0:1, 128 * s : 128 * (s + 1)].rearrange("a b -> b a"),
                        inv_col,
                    )
                nc.gpsimd.collective_compute(
                    "AllGather",
                    ALU.bypass,
                    replica_groups=RG,
                    ins=[inv_in[:]],
                    outs=[inv_out[:, :]],
                )

                # ---- A2: kv_a: 4 ckv m-tiles + kpe + kpe2 ----
                cv_t = cvsb.tile([128, CV_T, SB], BF, tag="cv")
                cvn_t = cvsb.tile([128, CV_T, SB], BF, tag="cvn")
                ssc = ssq2_ps.tile([128, SB], F32, tag="ssc")
                pe_ps = []
                for m in range(6):
                    mp = 128 if m < 4 else 64
                    col = slice(128 * m, 128 * m + 128) if m < 4 else \
                        slice(512 + 64 * (m - 4), 512 + 64 * (m - 3))
                    ps = st_ps.tile([mp, SB], F32, tag="stps")
                    for k in range(D_T):
                        nc.tensor.matmul(
                            ps,
                            kva_w[:, k, col],
                            hb_t[:, k, :],
                            start=(k == 0),
                            stop=(k == D_T - 1),
                        )
                    if m < 4:
                        nc.vector.tensor_copy(cv_t[:, m, :], ps)
                        sq = sqp.tile([128, SB], BF, tag="sq")
                        nc.scalar.activation(out=sq, in_=ps, func=AF.Square)
                        nc.tensor.matmul(
                            ssc, ones_t, sq, start=(m == 0), stop=(m == CV_T - 1)
                        )
                    else:
                        pe_ps.append(ps)

                bc2 = nrm.tile([128, SB], F32, tag="bc2")
                nc.scalar.activation(
                    out=bc2, in_=ssc, func=AF.Sqrt, scale=1.0 / KV_LORA, bias=eps_t
                )
                nc.vector.reciprocal(bc2, bc2)
                for i in range(CV_T):
                    nc.vector.tensor_mul(cvn_t[:, i, :], cv_t[:, i, :], bc2)
                    nc.gpsimd.dma_start(kv_in[128 * i : 128 * (i + 1), :], cvn_t[:, i, :])
                t1 = pet.tile([128, SB], F32, tag="t1")
                t2 = pet.tile([128, SB], F32, tag="t2")
                kper_sh = pet.tile([64, SB], BF, tag="kpersh")
                nc.vector.tensor_mul(t1[0:64, :], pe_ps[0], cos_c[0:64, :])
                nc.vector.tensor_mul(t2[0:64, :], pe_ps[1], sin_c[0:64, :])
                nc.vector.tensor_add(kper_sh[:, :], t1[0:64, :], t2[0:64, :])
                nc.gpsimd.dma_start(kv_in[512:576, :], kper_sh[:, :])
                nc.gpsimd.collective_compute(
                    "AllGather",
                    ALU.bypass,
                    replica_groups=RG,
                    ins=[kv_in[:]],
                    outs=[kv_out[:, :]],
                )

            # ------------- phase B: fused q projection over all chunks -------------
            with tc.tile_pool(name="qfw", bufs=1) as qfw, \
                 tc.tile_pool(name="hx", bufs=2) as hx, \
                 tc.tile_pool(name="csp", bufs=2) as csp, \
                 tc.tile_pool(name="bcp", bufs=2) as bcp, \
                 tc.tile_pool(name="pet2", bufs=1) as pet2, \
                 tc.tile_pool(name="qt_ps", bufs=5, space="PSUM") as qt_ps, \
                 tc.tile_pool(name="bc_ps", bufs=2, space="PSUM") as bc_ps:

                qf_w = qfw.tile([128, D_T, 512], BF, tag="qfw")
                for k in range(D_T):
                    eng = nc.sync if k % 2 == 0 else nc.scalar
                    eng.dma_start(out=qf_w[:, k, :], in_=qfT[128 * k : 128 * (k + 1), :])
                for b in range(N_CORES):
                    nc.sync.dma_start(
                        out=inv_sb[0:1, SB * b : SB * (b + 1)], in_=inv_out[b : b + 1, :]
                    )

                for c in range(NCHUNK):
                    cs = slice(NQ * c, NQ * (c + 1))
                    h_t = hx.tile([128, D_T, NQ], BF, tag="h")
                    for k in range(D_T):
                        eng = nc.sync if k % 2 == 0 else nc.scalar
                        eng.dma_start(out=h_t[:, k, :], in_=hT[128 * k : 128 * (k + 1), cs])
                    cos_c = csp.tile([128, NQ], F32, tag="cosc")
                    sin_c = csp.tile([128, NQ], F32, tag="sinc")
                    nc.sync.dma_start(out=cos_c, in_=cosb[:, cs])
                    nc.sync.dma_start(out=sin_c, in_=sinb[:, cs])

                    qt_tiles = []
                    for b in range(4):
                        ps = qt_ps.tile([128, NQ], F32, tag="qtps")
                        for k in range(D_T):
                            nc.tensor.matmul(
                                ps,
                                qf_w[:, k, 128 * b : 128 * (b + 1)],
                                h_t[:, k, :],
                                start=(k == 0),
                                stop=(k == D_T - 1),
                            )
                        qt_tiles.append(ps)
                    # per-position 1/rms arrives via the inv AllGather; broadcast
                    # the row across partitions with a K=1 matmul
                    bc_p = bc_ps.tile([128, NQ], F32, tag="bcps")
                    nc.tensor.matmul(
                        bc_p, ones_f, inv_sb[0:1, cs], start=True, stop=True
                    )
                    bc = bcp.tile([128, NQ], F32, tag="bc")
                    nc.scalar.activation(out=bc, in_=bc_p, func=AF.Copy)
                    nc.vector.tensor_mul(qn_T[0][:, cs], qt_tiles[0], bc)
                    nc.vector.tensor_mul(qn_T[1][:, cs], qt_tiles[1], bc)
                    t1 = pet2.tile([128, NQ], F32, tag="t1")
                    t2 = pet2.tile([128, NQ], F32, tag="t2")
                    nc.vector.tensor_mul(t1, qt_tiles[2], cos_c)
                    nc.vector.tensor_mul(t2, qt_tiles[3], sin_c)
                    nc.vector.tensor_add(t1, t1, t2)
                    nc.vector.tensor_mul(qpeP[:, cs], t1, bc)

            # ---------------- phase B2: unpack gathered kv + kv_b projections ----------------
            with tc.tile_pool(name="persist2", bufs=1) as persist2:
                kn_T = [persist2.tile([128, S], BF, tag=f"knT{h}", name=f"knT{h}") for h in range(HPC)]
                v_sb = [persist2.tile([128, S], BF, tag=f"v{h}", name=f"v{h}") for h in range(HPC)]
                o_T = [persist2.tile([128, S], BF, tag=f"oT{h}", name=f"oT{h}") for h in range(HPC)]
                with tc.tile_pool(name="kbw", bufs=1) as kbw, \
                     tc.tile_pool(name="kn_ps", bufs=2, space="PSUM") as kn_ps, \
                     tc.tile_pool(name="v_ps", bufs=3, space="PSUM") as v_ps:
                    for b in range(N_CORES):
                        bs = slice(SB * b, SB * (b + 1))
                        for ct in range(CV_T):
                            nc.sync.dma_start(
                                out=ckvn[ct][:, bs],
                                in_=kv_out[576 * b + 128 * ct : 576 * b + 128 * (ct + 1), :],
                            )
                        nc.scalar.dma_start(
                            out=kperLo[0:64, bs], in_=kv_out[576 * b + 512 : 576 * b + 576, :]
                        )
                        nc.scalar.dma_start(
                            out=kperHi[64:128, bs], in_=kv_out[576 * b + 512 : 576 * b + 576, :]
                        )
                    kb_w = kbw.tile([128, CV_T, 256], BF, tag="kbw")
                    vb_w = kbw.tile([128, CV_T, 256], BF, tag="vbw")
                    for ct in range(CV_T):
                        nc.sync.dma_start(out=kb_w[:, ct, :], in_=kbT[128 * ct : 128 * (ct + 1), :])
                        nc.sync.dma_start(out=vb_w[:, ct, :], in_=vbT[128 * ct : 128 * (ct + 1), :])
                    for h in range(HPC):
                        hs = slice(128 * h, 128 * (h + 1))
                        for c in range(NCHUNK):
                            cs = slice(NQ * c, NQ * (c + 1))
                            ps = kn_ps.tile([128, NQ], F32, tag="knps")
                            for ct in range(CV_T):
                                nc.tensor.matmul(
                                    ps,
                                    kb_w[:, ct, hs],
                                    ckvn[ct][:, cs],
                                    start=(ct == 0),
                                    stop=(ct == CV_T - 1),
                                )
                            nc.vector.tensor_copy(kn_T[h][:, cs], ps)
                    # v for both heads per matmul (moving free = 256)
                    for kt in range(KT):
                        ks = slice(128 * kt, 128 * (kt + 1))
                        ps = v_ps.tile([128, 2 * VD], F32, tag="vps")
                        for ct in range(CV_T):
                            nc.tensor.matmul(
                                ps,
                                ckvn[ct][:, ks],
                                vb_w[:, ct, :],
                                start=(ct == 0),
                                stop=(ct == CV_T - 1),
                            )
                        nc.vector.tensor_copy(v_sb[0][:, ks], ps[:, 0:VD])
                        nc.vector.tensor_copy(v_sb[1][:, ks], ps[:, VD : 2 * VD])

                # ---------------- phase C: attention ----------------
                mskp_cm = tc.tile_pool(name="mskp", bufs=1)
                oww_cm = tc.tile_pool(name="oww", bufs=1)
                mskp = mskp_cm.__enter__()
                oww = oww_cm.__enter__()
                with tc.tile_pool(name="pp", bufs=6) as pp, \
                     tc.tile_pool(name="ep", bufs=3) as ep, \
                     tc.tile_pool(name="rvp", bufs=2) as rvp, \
                     tc.tile_pool(name="ostg", bufs=4) as ostg, \
                     tc.tile_pool(name="s_ps", bufs=3, space="PSUM") as s_ps, \
                     tc.tile_pool(name="rs_ps", bufs=2, space="PSUM") as rs_ps, \
                     tc.tile_pool(name="o_ps", bufs=2, space="PSUM") as o_ps, \
                     tc.tile_pool(name="out_ps", bufs=1, space="PSUM") as out_ps:
                    mask_s = mskp.tile([128, 1024], BF, tag="mask")
                    nc.sync.dma_start(out=mask_s, in_=maskb[:, :])
                    ow_t = oww.tile([128, HPC, D], BF, tag="oww")
                    for j in range(HPC):
                        nc.sync.dma_start(out=ow_t[:, j, :], in_=owT[128 * j : 128 * (j + 1), :])
                    for c in range(NCHUNK):
                        cs = slice(NQ * c, NQ * (c + 1))
                        nkt = 4 * (c + 1)
                        for h in range(HPC):
                            kper_h = kperLo if h == 0 else kperHi
                            rs = rs_ps.tile([128, NQ], F32, tag="rs")
                            op = o_ps.tile([128, NQ], F32, tag="op")
                            for kt in range(nkt):
                                ks = slice(128 * kt, 128 * (kt + 1))
                                i = kt - 4 * c
                                lo = 128 * i if i > 0 else 0  # valid q-subrange start
                                qs = slice(NQ * c + lo, NQ * (c + 1))
                                vs = slice(lo, NQ)
                                sp = s_ps.tile([128, NQ], F32, tag="sp")
                                nc.tensor.matmul(
                                    sp[:, vs], kn_T[h][:, ks], qn_T[h][:, qs],
                                    start=True, stop=False,
                                )
                                nc.tensor.matmul(
                                    sp[:, vs], kper_h[:, ks], qpeP[:, qs],
                                    start=False, stop=True,
                                )
                                p_t = pp.tile([128, NQ], BF, tag="p")
                                if kt >= 4 * c:
                                    e_t = ep.tile([128, NQ], BF, tag="e")
                                    nc.scalar.activation(out=e_t[:, vs], in_=sp[:, vs], func=AF.Exp)
                                    nc.vector.tensor_mul(
                                        p_t[:, vs], e_t[:, vs],
                                        mask_s[:, 384 : 896 - lo],
                                    )
                                else:
                                    nc.scalar.activation(out=p_t[:, vs], in_=sp[:, vs], func=AF.Exp)
                                nc.tensor.matmul(
                                    rs[:, vs], ones_t, p_t[:, vs],
                                    start=(kt == 0), stop=(kt == nkt - 1),
                                )
                                nc.tensor.matmul(
                                    op[:, vs],
                                    v_sb[h][:, ks],
                                    p_t[:, vs],
                                    start=(kt == 0), stop=(kt == nkt - 1),
                                )
                            rv = rvp.tile([128, NQ], F32, tag="rv")
                            nc.vector.reciprocal(rv, rs)
                            nc.vector.tensor_mul(o_T[h][:, cs], op, rv)
                        # o_proj for this chunk's 4 s-tiles (both heads now done;
                        # last chunk handled in a post-phase with deeper PSUM)
                        for si in range(4 * c, 4 * (c + 1) if c < NCHUNK - 1 else 4 * c):
                            ss = slice(128 * si, 128 * (si + 1))
                            for nch in range(NCHUNK):
                                ns = slice(NQ * nch, NQ * (nch + 1))
                                ps = out_ps.tile([128, NQ], F32, tag="outps")
                                for j in range(HPC):
                                    nc.tensor.matmul(
                                        ps,
                                        o_T[j][:, ss],
                                        ow_t[:, j, ns],
                                        start=(j == 0),
                                        stop=(j == HPC - 1),
                                    )
                                stg = ostg.tile([128, NQ], F32, tag="ostg")
                                nc.scalar.activation(out=stg, in_=ps, func=AF.Copy)
                                nc.sync.dma_start(out=out[ss, ns], in_=stg)
                # ---------------- final chunk o_proj ----------------
                with tc.tile_pool(name="ostg2", bufs=4) as ostg2, \
                     tc.tile_pool(name="out2_ps", bufs=4, space="PSUM") as out2_ps:
                    for si in range(4 * (NCHUNK - 1), 4 * NCHUNK):
                        ss = slice(128 * si, 128 * (si + 1))
                        for nch in range(NCHUNK):
                            ns = slice(NQ * nch, NQ * (nch + 1))
                            ps = out2_ps.tile([128, NQ], F32, tag="out2ps")
                            for j in range(HPC):
                                nc.tensor.matmul(
                                    ps,
                                    o_T[j][:, ss],
                                    ow_t[:, j, ns],
                                    start=(j == 0),
                                    stop=(j == HPC - 1),
                                )
                            stg = ostg2.tile([128, NQ], F32, tag="ostg2")
                            nc.scalar.activation(out=stg, in_=ps, func=AF.Copy)
                            nc.sync.dma_start(out=out[ss, ns], in_=stg)
                oww_cm.__exit__(None, None, None)
                mskp_cm.__exit__(None, None, None)

    _split_waits(nc)
    return nc


# ----------------------------------------------------------------------------
# entry point
# ----------------------------------------------------------------------------

def kernel(**inputs):
    global LAST_RESULTS
    shared, per_core = _prep_inputs(inputs)
    if "nc" not in _CACHE:
        _CACHE["nc"] = _build_nc()
    nc = _CACHE["nc"]
    in_maps = []
    for c in range(N_CORES):
        m = {
            "hT": shared["hT"],
            "qaT": shared["qaT"],
            "kvaT": shared["kvaT"],
            "cosb": shared["cosb"],
            "sinb": shared["sinb"],
            "maskb": shared["maskb"],
            "qfT": per_core[c]["qfT"],
            "kbT": per_core[c]["kbT"],
            "vbT": per_core[c]["vbT"],
            "owT": per_core[c]["owT"],
            "hb": per_core[c]["hb"],
            "cosk": per_core[c]["cosk"],
            "sink": per_core[c]["sink"],
        }
        in_maps.append(m)
    res = run_bass_kernel_spmd(nc, in_maps, core_ids=list(range(N_CORES)))
    LAST_RESULTS = res
    out = np.zeros((S, D), dtype=np.float32)
    for r in res.results:
        out += r["out"]
    return out.reshape(B, S, D)


# revision 11
# speedup vs baseline: 1.4217x; 1.0197x over previous
"""DeepseekV3 MLA attention (B=1, S=2048, D=2048, H=16) on 8 trn2 NeuronCores.

Strategy (v4 = v3 + batched DMA layouts + cheap reciprocals):
  - the q_b@q_a product is FUSED on the host per core's 2 heads, so each core
    computes q directly from hidden states (contraction D=2048) and nobody
    materializes the full q_a activation;
  - the q rmsnorm statistic is sharded: core c computes q_a only for seq
    positions [256c, 256c+256) (seq on PSUM partitions so the square-sum
    runs on the vector engine), reduces to a 1/rms row, and an 8-core
    AllGather ([1,256] f32 per core) distributes it;
  - kv_a is sharded the same way: core c computes ckv for its 256 positions,
    normalizes + ropes locally, and an AllGather ([576,256] bf16 per core)
    distributes the shared K/V latents to every core;
  - each core owns 2 heads: kv_b, causal flash-style attention (no max
    subtraction -- logits are O(1) here), and its slice of o_proj, producing
    a partial [S, D] output; host sums the 8 partials.

Weights are shipped in PE-tile-major layout ([128, k, cols]) so each loads
with one or two DMA descript['s] -- DMA triggers cost ~0.6us of issuing-engine
time each, so count matters.  The scalar engine queue is kept free of DMA
triggers during phase A so the rms chain (Square/Sqrt) isn't stuck behind
them (that delayed the inv AllGather by ~40us in v3).
"""

import numpy as np
import ml_dtypes

import concourse.bass as bass
import concourse.mybir as mybir
import concourse.tile as tile
from concourse.bass_utils import run_bass_kernel_spmd

BF16 = ml_dtypes.bfloat16
F32 = mybir.dt.float32
BF = mybir.dt.bfloat16

B, S, D = 1, 2048, 2048
H = 16
N_CORES = 8
HPC = H // N_CORES  # heads per core = 2
Q_LORA = 1536
KV_LORA = 512
NOPE = 128
ROPE = 64
VD = 128
QHD = NOPE + ROPE  # 192
THETA = 50000.0
EPS = 1e-6
SCALE = QHD ** (-0.5)

NQ = 512            # q-chunk (matmul free dim)
NCHUNK = S // NQ    # 4
KT = S // 128       # 16 k-tiles
SB = S // N_CORES   # 256: per-core seq block for the sharded projections
AF = mybir.ActivationFunctionType
ALU = mybir.AluOpType

LAST_RESULTS = None
_CACHE = {}


def _tiled(w):
    """[R, C] -> [128, R//128, C] PE-tile-major (partition, k-tile, col)."""
    r, c = w.shape
    return np.ascontiguousarray(w.reshape(r // 128, 128, c).transpose(1, 0, 2))


# ----------------------------------------------------------------------------
# host-side weight preparation
# ----------------------------------------------------------------------------

def _deint_perm():
    # deinterleave: out[j] = in[2j] (j<32), in[2(j-32)+1] (j>=32)
    p = np.empty(ROPE, dtype=np.int64)
    p[:32] = 2 * np.arange(32)
    p[32:] = 2 * np.arange(32) + 1
    return p


def _rope_tables(position_ids):
    pos = np.asarray(position_ids).reshape(-1).astype(np.float32)  # [S]
    inv_freq = (1.0 / (THETA ** (np.arange(0, ROPE, 2, dtype=np.float32) / ROPE)))
    freqs = np.outer(pos, inv_freq)  # [S, 32]
    cos32 = np.cos(freqs).T.astype(np.float32)  # [32, S]
    sin32 = np.sin(freqs).T.astype(np.float32)
    cos128 = np.tile(cos32, (4, 1))  # [128, S]
    sin128 = np.tile(sin32, (4, 1))
    return cos128, sin128


def _causal_mask_big():
    # M[dk, u] = 1 if u >= dk + 384 ; slice [:, 384-128*i : 896-128*i]
    # gives the diagonal-block mask indicator(dq >= dk + 128*i)
    dk = np.arange(128)[:, None]
    u = np.arange(1024)[None, :]
    return (u >= dk + 384).astype(BF16)


def _prep_inputs(inputs):
    hidden = np.asarray(inputs["hidden_states"], dtype=np.float32)[0]  # [S, D]
    position_ids = np.asarray(inputs["position_ids"])
    q_a_w = np.asarray(inputs["q_a_w"], dtype=np.float32)        # [1536, D]
    q_a_ln_w = np.asarray(inputs["q_a_ln_w"], dtype=np.float32)  # [1536]
    q_b_w = np.asarray(inputs["q_b_w"], dtype=np.float32)        # [H*192, 1536]
    kv_a_w = np.asarray(inputs["kv_a_w"], dtype=np.float32)      # [576, D]
    kv_a_ln_w = np.asarray(inputs["kv_a_ln_w"], dtype=np.float32)  # [512]
    kv_b_w = np.asarray(inputs["kv_b_w"], dtype=np.float32)      # [H*256, 512]
    o_w = np.asarray(inputs["o_w"], dtype=np.float32)            # [D, H*128]

    dp = _deint_perm()
    dps = dp[(np.arange(ROPE) ^ 32)]          # source index for the swapped term
    sgn = np.where(np.arange(ROPE) < 32, -1.0, 1.0).astype(np.float32)[:, None]

    hT = np.ascontiguousarray(hidden.T).astype(BF16)                   # [D, S]
    shared = {}
    shared["h3"] = _tiled(hT)                                          # [128, 16, S]
    shared["qa3"] = _tiled(np.ascontiguousarray(q_a_w.T).astype(BF16))  # [128,16,1536]

    # kv_a columns: [ckv 512 | kpe 64 (deint) | kpe2 64 (swap+sign)]
    kva_cols = np.concatenate(
        [kv_a_w[:KV_LORA], kv_a_w[KV_LORA + dp], sgn * kv_a_w[KV_LORA + dps]], axis=0
    )  # [640, D]
    shared["kva3"] = _tiled(np.ascontiguousarray(kva_cols.T).astype(BF16))  # [128,16,640]

    cos128, sin128 = _rope_tables(position_ids)
    shared["cosb"] = cos128
    shared["sinb"] = sin128
    shared["maskb"] = _causal_mask_big()

    # q_b with ln + scale folded
    qb = q_b_w * q_a_ln_w[None, :] * SCALE  # [H*192, 1536]
    qb = qb.reshape(H, QHD, Q_LORA)
    kvb = (kv_b_w * kv_a_ln_w[None, :]).reshape(H, NOPE + VD, KV_LORA)

    per_core = []
    for c in range(N_CORES):
        h0, h1 = HPC * c, HPC * c + 1
        nope0 = qb[h0, :NOPE]            # [128, 1536]
        nope1 = qb[h1, :NOPE]
        peP = np.concatenate([qb[h0, NOPE + dp], qb[h1, NOPE + dp]], axis=0)  # [128,...]
        pe2P = np.concatenate(
            [sgn * qb[h0, NOPE + dps], sgn * qb[h1, NOPE + dps]], axis=0
        )
        qb_cols = np.concatenate([nope0, nope1, peP, pe2P], axis=0)  # [512, 1536]
        # fuse q_b @ q_a: [512, 1536] @ [1536, D] -> [512, D]
        qf_cols = qb_cols @ q_a_w  # fp32
        kb_cols = np.concatenate([kvb[h0, :NOPE], kvb[h1, :NOPE]], axis=0)  # [256, 512]
        vb_cols = np.concatenate([kvb[h0, NOPE:], kvb[h1, NOPE:]], axis=0)  # [256, 512]
        o_slice = o_w[:, VD * h0 : VD * (h1 + 1)]  # [D, 256]
        blk = slice(SB * c, SB * (c + 1))
        per_core.append(
            {
                "qf3": _tiled(np.ascontiguousarray(qf_cols.T).astype(BF16)),  # [128,16,512]
                "kb3": _tiled(np.ascontiguousarray(kb_cols.T).astype(BF16)),  # [128,4,256]
                "vb3": _tiled(np.ascontiguousarray(vb_cols.T).astype(BF16)),  # [128,4,256]
                "ow3": _tiled(np.ascontiguousarray(o_slice.T).astype(BF16)),  # [128,2,D]
                "hb3": np.ascontiguousarray(shared["h3"][:, :, blk]),         # [128,16,256]
                "cosk": np.ascontiguousarray(cos128[:, blk]),          # [128, 256]
                "sink": np.ascontiguousarray(sin128[:, blk]),          # [128, 256]
            }
        )
    return shared, per_core


# ----------------------------------------------------------------------------
# numpy simulation of the device program (for host-side validation)
# ----------------------------------------------------------------------------

def _untile(w3):
    p, k, c = w3.shape
    return w3.transpose(1, 0, 2).reshape(p * k, c)


def _sim_phase_a(shared, pc):
    """Per-core sharded projections: returns (inv_blk [SB], kv_shard [576, SB])."""
    bf = lambda x: x.astype(BF16).astype(np.float32)
    hb = _untile(pc["hb3"]).astype(np.float32)    # [D, SB]
    qaT = _untile(shared["qa3"]).astype(np.float32)   # [D, 1536]
    kvaT = _untile(shared["kva3"]).astype(np.float32)  # [D, 640]
    cos = pc["cosk"]                              # [128, SB]
    sin = pc["sink"]

    ckvT = kvaT.T @ hb                            # [640, SB]
    ckvb = bf(ckvT[:KV_LORA])
    ssc = (bf(ckvb * ckvb)).sum(axis=0)
    invc = 1.0 / np.sqrt(ssc / KV_LORA + EPS)
    ckvn = bf(ckvb * invc)                        # [512, SB]
    kpe, kpe2 = ckvT[512:576], ckvT[576:640]
    kper = bf(kpe * cos[0:64] + kpe2 * sin[0:64])  # [64, SB]
    kv_shard = np.concatenate([ckvn, kper], axis=0)  # [576, SB]

    qaTx = qaT.T @ hb                             # [1536, SB] (fp32 PSUM values)
    ssq = bf(qaTx * qaTx).sum(axis=0)             # bf16 squares, fp32 reduce
    inv = 1.0 / np.sqrt(ssq / Q_LORA + EPS)       # [SB]
    return inv.astype(np.float32), kv_shard.astype(BF16)


def _sim_core(shared, pc, inv, ckvn, kper):
    bf = lambda x: x.astype(BF16).astype(np.float32)
    hT = _untile(shared["h3"]).astype(np.float32)  # [D, S]
    cos = shared["cosb"]                          # [128, S]
    sin = shared["sinb"]
    qfT = _untile(pc["qf3"]).astype(np.float32)   # [D, 512]
    kbT = _untile(pc["kb3"]).astype(np.float32)   # [512, 256]
    vbT = _untile(pc["vb3"]).astype(np.float32)   # [512, 256]
    owT = _untile(pc["ow3"]).astype(np.float32)   # [256, D]

    qT = qfT.T @ hT                               # [512, S]
    qn0 = bf(qT[0:128] * inv)
    qn1 = bf(qT[128:256] * inv)
    pe, pe2 = qT[256:384], qT[384:512]
    qpe = bf((pe * cos + pe2 * sin) * inv)        # [128, S] packed (h0;h1)

    ckvn = ckvn.astype(np.float32)                # [512, S]
    kper = kper.astype(np.float32)                # [64, S]

    out = np.zeros((S, D), dtype=np.float32)
    for j in range(HPC):
        knT = bf(kbT[:, 128 * j : 128 * (j + 1)].T @ ckvn)   # [128, S]
        v = bf(ckvn.T @ vbT[:, 128 * j : 128 * (j + 1)])     # [S, 128]
        qn = qn0 if j == 0 else qn1
        qp = qpe[64 * j : 64 * (j + 1)]
        scores = knT.T @ qn + kper.T @ qp         # [S(k), S(q)] -> st[k, q]
        kidx = np.arange(S)[:, None]
        qidx = np.arange(S)[None, :]
        p = np.exp(scores) * (kidx <= qidx)
        p = bf(p)
        rs = p.sum(axis=0)                        # [q]
        oT = (v.T @ p)                            # [128, q]
        oT = bf(oT * (1.0 / rs))
        out += oT.T @ owT[128 * j : 128 * (j + 1)]
    return out


def sim(inputs):
    shared, per_core = _prep_inputs(inputs)
    invs, shards = [], []
    for c in range(N_CORES):
        inv, shard = _sim_phase_a(shared, per_core[c])
        invs.append(inv)
        shards.append(shard)
    inv = np.concatenate(invs)                    # [S]
    gathered = np.concatenate(shards, axis=1)     # [576, S]
    ckvn, kper = gathered[:KV_LORA], gathered[KV_LORA:]
    out = np.zeros((S, D), dtype=np.float32)
    for c in range(N_CORES):
        out += _sim_core(shared, per_core[c], inv, ckvn, kper)
    return out.reshape(B, S, D)


# ----------------------------------------------------------------------------
# bass program
# ----------------------------------------------------------------------------

def _split_waits(nc, max_waits=1):
    """This walrus build accepts at most one sem wait per instruction; hoist
    excess waits onto pure-wait EventSemaphore carriers just before it."""
    n_new = 0
    for f in nc.m.functions:
        for blk in f.blocks:
            new_insts = []
            for inst in blk.instructions:
                si = getattr(inst, "sync_info", None)
                waits = list(si.on_wait) if (si is not None and si.on_wait) else []
                if len(waits) > max_waits:
                    extra, keep = waits[:-max_waits], waits[-max_waits:]
                    for w in extra:
                        n_new += 1
                        carrier = mybir.InstEventSemaphore(
                            name=f"ws-{n_new}-{inst.name}",
                            engine=inst.engine,
                            ins=[],
                            outs=[],
                            sync_info=mybir.SyncInfo(on_wait=[w], on_update=[]),
                        )
                        nc.register_instruction(carrier, overwrite=True)
                        new_insts.append(carrier)
                    si.on_wait = keep
                new_insts.append(inst)
            blk.instructions = new_insts
    return n_new


def _build_nc():
    nc = bass.Bass(num_devices=N_CORES)
    h3 = nc.dram_tensor("h3", [128, KT, S], BF, kind="ExternalInput")
    hb3 = nc.dram_tensor("hb3", [128, KT, SB], BF, kind="ExternalInput")
    qa3 = nc.dram_tensor("qa3", [128, KT, Q_LORA], BF, kind="ExternalInput")
    kva3 = nc.dram_tensor("kva3", [128, KT, 640], BF, kind="ExternalInput")
    qf3 = nc.dram_tensor("qf3", [128, KT, 512], BF, kind="ExternalInput")
    kb3 = nc.dram_tensor("kb3", [128, 4, 256], BF, kind="ExternalInput")
    vb3 = nc.dram_tensor("vb3", [128, 4, 256], BF, kind="ExternalInput")
    ow3 = nc.dram_tensor("ow3", [128, HPC, D], BF, kind="ExternalInput")
    cosb = nc.dram_tensor("cosb", [128, S], F32, kind="ExternalInput")
    sinb = nc.dram_tensor("sinb", [128, S], F32, kind="ExternalInput")
    cosk = nc.dram_tensor("cosk", [128, SB], F32, kind="ExternalInput")
    sink = nc.dram_tensor("sink", [128, SB], F32, kind="ExternalInput")
    maskb = nc.dram_tensor("maskb", [128, 1024], BF, kind="ExternalInput")
    out = nc.dram_tensor("out", [S, D], F32, kind="ExternalOutput")
    # collective outputs live in the Shared scratchpad (faster HBM-HBM path)
    kv_out = nc.dram_tensor("kv_out_sh", [576 * N_CORES, SB], BF, addr_space="Shared")
    inv_out = nc.dram_tensor("inv_out_sh", [N_CORES, SB], F32, addr_space="Shared")

    QL_T = Q_LORA // 128  # 12
    D_T = D // 128        # 16
    CV_T = KV_LORA // 128  # 4
    RG = [list(range(N_CORES))]

    with tile.TileContext(nc) as tc:
        with tc.tile_pool(name="persist1", bufs=1) as persist1, \
             tc.tile_pool(name="dram", bufs=1, space="DRAM") as dram:
            ones_t = persist1.tile([128, 128], BF, tag="ones")
            ones_f = persist1.tile([1, 128], F32, tag="onesf")
            eps_t = persist1.tile([128, 1], F32, tag="eps")
            nc.vector.memset(eps_t, EPS)
            nc.vector.memset(ones_t, 1.0)
            nc.vector.memset(ones_f, 1.0)
            qn_T = [persist1.tile([128, S], BF, tag=f"qnT{h}", name=f"qnT{h}") for h in range(HPC)]
            qpeP = persist1.tile([128, S], BF, tag="qpeP")
            ckvn = [persist1.tile([128, S], BF, tag=f"ckvn{i}", name=f"ckvn{i}") for i in range(CV_T)]
            kperLo = persist1.tile([128, S], BF, tag="kperLo")
            kperHi = persist1.tile([128, S], BF, tag="kperHi")
            inv_sb = persist1.tile([1, S], F32, tag="invsb")
            nc.vector.memset(kperLo[64:128, :], 0.0)
            nc.vector.memset(kperHi[0:64, :], 0.0)

            kv_in = dram.tile([576, SB], BF, tag="kvin")
            inv_in = dram.tile([1, SB], F32, tag="invin")

            # ------------- phase A: sharded q_a rms stat + kv_a for this core's block -------------
            with tc.tile_pool(name="qaw", bufs=1) as qaw, \
                 tc.tile_pool(name="kvw", bufs=1) as kvw, \
                 tc.tile_pool(name="hbx", bufs=1) as hbx, \
                 tc.tile_pool(name="cskp", bufs=1) as cskp, \
                 tc.tile_pool(name="cvsb", bufs=1) as cvsb, \
                 tc.tile_pool(name="sq", bufs=3) as sqp, \
                 tc.tile_pool(name="acc", bufs=1) as accp, \
                 tc.tile_pool(name="nrm", bufs=2) as nrm, \
                 tc.tile_pool(name="pet", bufs=1) as pet:

                hb_t = hbx.tile([128, D_T, SB], BF, tag="hb")
                qa_w = qaw.tile([128, D_T, Q_LORA], BF, tag="qaw")
                kva_w = kvw.tile([128, D_T, 640], BF, tag="kvw")
                nc.sync.dma_start(out=hb_t[:, :, :], in_=hb3[:, :, :])
                for mb in range(3):
                    mcs = slice(512 * mb, 512 * (mb + 1))
                    nc.sync.dma_start(out=qa_w[:, :, mcs], in_=qa3[:, :, mcs])
                nc.sync.dma_start(out=kva_w[:, :, 0:320], in_=kva3[:, :, 0:320])
                nc.sync.dma_start(out=kva_w[:, :, 320:640], in_=kva3[:, :, 320:640])
                cos_c = cskp.tile([128, SB], F32, tag="coskc")
                sin_c = cskp.tile([128, SB], F32, tag="sinkc")
                nc.sync.dma_start(out=cos_c, in_=cosk[:, :])
                nc.sync.dma_start(out=sin_c, in_=sink[:, :])

                # ---- A1: q_a squares, seq on partitions; DVE row-reduce ----
                with tc.tile_pool(name="qa_ps", bufs=1, space="PSUM") as qa_ps:
                    pss = []
                    for s in range(2):
                        for mb in range(3):
                            ps = qa_ps.tile([128, 512], F32, tag=f"qaps{s}{mb}")
                            for k in range(D_T):
                                nc.tensor.matmul(
                                    ps,
                                    hb_t[:, k, 128 * s : 128 * (s + 1)],
                                    qa_w[:, k, 512 * mb : 512 * (mb + 1)],
                                    start=(k == 0),
                                    stop=(k == D_T - 1),
                                )
                            pss.append(ps)
                    accs = []
                    for i, ps in enumerate(pss):
                        sqd = sqp.tile([128, 512], BF, tag="sqd")
                        nc.scalar.activation(out=sqd, in_=ps, func=AF.Square)
                        acc = accp.tile([128, 1], F32, tag=f"acc{i}")
                        nc.vector.reduce_sum(
                            out=acc, in_=sqd, axis=mybir.AxisListType.X
                        )
                        accs.append(acc)
                    for s in range(2):
                        a = accs[3 * s : 3 * s + 3]
                        nc.vector.tensor_add(a[0], a[0], a[1])
                        nc.vector.tensor_add(a[0], a[0], a[2])
                        inv_col = nrm.tile([128, 1], F32, tag=f"invc{s}")
                        nc.scalar.activation(
                            out=inv_col, in_=a[0], func=AF.Sqrt,
                            scale=1.0 / Q_LORA, bias=eps_t,
                        )
                        nc.vector.reciprocal(inv_col, inv_col)
                        nc.gpsimd.dma_start(
                            inv_in[0:1, 128 * s : 128 * (s + 1)].rearrange("a b -> b a"),
                            inv_col,
                        )
                nc.gpsimd.collective_compute(
                    "AllGather",
                    ALU.bypass,
                    replica_groups=RG,
                    ins=[inv_in[:]],
                    outs=[inv_out[:, :]],
                )

                # ---- A2: kv_a: 4 ckv m-tiles + kpe + kpe2 ----
                with tc.tile_pool(name="st_ps", bufs=3, space="PSUM") as st_ps, \
                     tc.tile_pool(name="ssq2_ps", bufs=1, space="PSUM") as ssq2_ps:
                    cv_t = cvsb.tile([128, CV_T, SB], BF, tag="cv")
                    cvn_t = cvsb.tile([128, CV_T, SB], BF, tag="cvn")
                    ssc = ssq2_ps.tile([128, SB], F32, tag="ssc")
                    pe_ps = []
                    for m in range(6):
                        mp = 128 if m < 4 else 64
                        col = slice(128 * m, 128 * m + 128) if m < 4 else \
                            slice(512 + 64 * (m - 4), 512 + 64 * (m - 3))
                        ps = st_ps.tile([mp, SB], F32, tag="stps")
                        for k in range(D_T):
                            nc.tensor.matmul(
                                ps,
                                kva_w[:, k, col],
                                hb_t[:, k, :],
                                start=(k == 0),
                                stop=(k == D_T - 1),
                            )
                        if m < 4:
                            nc.vector.tensor_copy(cv_t[:, m, :], ps)
                            sq = sqp.tile([128, SB], BF, tag="sq")
                            nc.scalar.activation(out=sq, in_=ps, func=AF.Square)
                            nc.tensor.matmul(
                                ssc, ones_t, sq, start=(m == 0), stop=(m == CV_T - 1)
                            )
                        else:
                            pe_ps.append(ps)

                    bc2 = nrm.tile([128, SB], F32, tag="bc2")
                    nc.scalar.activation(
                        out=bc2, in_=ssc, func=AF.Sqrt, scale=1.0 / KV_LORA, bias=eps_t
                    )
                    nc.vector.reciprocal(bc2, bc2)
                    for i in range(CV_T):
                        nc.vector.tensor_mul(cvn_t[:, i, :], cv_t[:, i, :], bc2)
                        nc.gpsimd.dma_start(kv_in[128 * i : 128 * (i + 1), :], cvn_t[:, i, :])
                    t1 = pet.tile([128, SB], F32, tag="t1")
                    t2 = pet.tile([128, SB], F32, tag="t2")
                    kper_sh = pet.tile([64, SB], BF, tag="kpersh")
                    nc.vector.tensor_mul(t1[0:64, :], pe_ps[0], cos_c[0:64, :])
                    nc.vector.tensor_mul(t2[0:64, :], pe_ps[1], sin_c[0:64, :])
                    nc.vector.tensor_add(kper_sh[:, :], t1[0:64, :], t2[0:64, :])
                    nc.gpsimd.dma_start(kv_in[512:576, :], kper_sh[:, :])
                nc.gpsimd.collective_compute(
                    "AllGather",
                    ALU.bypass,
                    replica_groups=RG,
                    ins=[kv_in[:]],
                    outs=[kv_out[:, :]],
                )

            # ------------- phase B: fused q projection over all chunks -------------
            with tc.tile_pool(name="qfw", bufs=1) as qfw, \
                 tc.tile_pool(name="hx", bufs=2) as hx, \
                 tc.tile_pool(name="csp", bufs=2) as csp, \
                 tc.tile_pool(name="bcp", bufs=2) as bcp, \
                 tc.tile_pool(name="pet2", bufs=1) as pet2, \
                 tc.tile_pool(name="qt_ps", bufs=5, space="PSUM") as qt_ps, \
                 tc.tile_pool(name="bc_ps", bufs=2, space="PSUM") as bc_ps:

                qf_w = qfw.tile([128, D_T, 512], BF, tag="qfw")
                nc.sync.dma_start(out=qf_w[:, :, 0:256], in_=qf3[:, :, 0:256])
                nc.sync.dma_start(out=qf_w[:, :, 256:512], in_=qf3[:, :, 256:512])
                for b in range(N_CORES):
                    nc.scalar.dma_start(
                        out=inv_sb[0:1, SB * b : SB * (b + 1)], in_=inv_out[b : b + 1, :]
                    )

                for c in range(NCHUNK):
                    cs = slice(NQ * c, NQ * (c + 1))
                    h_t = hx.tile([128, D_T, NQ], BF, tag="h")
                    nc.sync.dma_start(out=h_t[:, 0:8, :], in_=h3[:, 0:8, cs])
                    nc.sync.dma_start(out=h_t[:, 8:16, :], in_=h3[:, 8:16, cs])
                    cos_c = csp.tile([128, NQ], F32, tag="cosc")
                    sin_c = csp.tile([128, NQ], F32, tag="sinc")
                    nc.scalar.dma_start(out=cos_c, in_=cosb[:, cs])
                    nc.scalar.dma_start(out=sin_c, in_=sinb[:, cs])

                    qt_tiles = []
                    for b in range(4):
                        ps = qt_ps.tile([128, NQ], F32, tag="qtps")
                        for k in range(D_T):
                            nc.tensor.matmul(
                                ps,
                                qf_w[:, k, 128 * b : 128 * (b + 1)],
                                h_t[:, k, :],
                                start=(k == 0),
                                stop=(k == D_T - 1),
                            )
                        qt_tiles.append(ps)
                    # per-position 1/rms arrives via the inv AllGather; broadcast
                    # the row across partitions with a K=1 matmul
                    bc_p = bc_ps.tile([128, NQ], F32, tag="bcps")
                    nc.tensor.matmul(
                        bc_p, ones_f, inv_sb[0:1, cs], start=True, stop=True
                    )
                    bc = bcp.tile([128, NQ], F32, tag="bc")
                    nc.scalar.activation(out=bc, in_=bc_p, func=AF.Copy)
                    nc.vector.tensor_mul(qn_T[0][:, cs], qt_tiles[0], bc)
                    nc.vector.tensor_mul(qn_T[1][:, cs], qt_tiles[1], bc)
                    t1 = pet2.tile([128, NQ], F32, tag="t1")
                    t2 = pet2.tile([128, NQ], F32, tag="t2")
                    nc.vector.tensor_mul(t1, qt_tiles[2], cos_c)
                    nc.vector.tensor_mul(t2, qt_tiles[3], sin_c)
                    nc.vector.tensor_add(t1, t1, t2)
                    nc.vector.tensor_mul(qpeP[:, cs], t1, bc)

            # ---------------- phase B2: unpack gathered kv + kv_b projections ----------------
            with tc.tile_pool(name="persist2", bufs=1) as persist2:
                kn_T = [persist2.tile([128, S], BF, tag=f"knT{h}", name=f"knT{h}") for h in range(HPC)]
                v_sb = [persist2.tile([128, S], BF, tag=f"v{h}", name=f"v{h}") for h in range(HPC)]
                o_T = [persist2.tile([128, S], BF, tag=f"oT{h}", name=f"oT{h}") for h in range(HPC)]
                with tc.tile_pool(name="kbw", bufs=1) as kbw, \
                     tc.tile_pool(name="kn_ps", bufs=2, space="PSUM") as kn_ps, \
                     tc.tile_pool(name="v_ps", bufs=3, space="PSUM") as v_ps:
                    for b in range(N_CORES):
                        bs = slice(SB * b, SB * (b + 1))
                        eng = nc.sync if b % 2 == 0 else nc.scalar
                        for ct in range(CV_T):
                            eng.dma_start(
                                out=ckvn[ct][:, bs],
                                in_=kv_out[576 * b + 128 * ct : 576 * b + 128 * (ct + 1), :],
                            )
                        eng.dma_start(
                            out=kperLo[0:64, bs], in_=kv_out[576 * b + 512 : 576 * b + 576, :]
                        )
                        eng.dma_start(
                            out=kperHi[64:128, bs], in_=kv_out[576 * b + 512 : 576 * b + 576, :]
                        )
                    kb_w = kbw.tile([128, CV_T, 256], BF, tag="kbw")
                    vb_w = kbw.tile([128, CV_T, 256], BF, tag="vbw")
                    nc.sync.dma_start(out=kb_w[:, :, :], in_=kb3[:, :, :])
                    nc.sync.dma_start(out=vb_w[:, :, :], in_=vb3[:, :, :])
                    for h in range(HPC):
                        hs = slice(128 * h, 128 * (h + 1))
                        for c in range(NCHUNK):
                            cs = slice(NQ * c, NQ * (c + 1))
                            ps = kn_ps.tile([128, NQ], F32, tag="knps")
                            for ct in range(CV_T):
                                nc.tensor.matmul(
                                    ps,
                                    kb_w[:, ct, hs],
                                    ckvn[ct][:, cs],
                                    start=(ct == 0),
                                    stop=(ct == CV_T - 1),
                                )
                            nc.vector.tensor_copy(kn_T[h][:, cs], ps)
                    # v for both heads per matmul (moving free = 256)
                    for kt in range(KT):
                        ks = slice(128 * kt, 128 * (kt + 1))
                        ps = v_ps.tile([128, 2 * VD], F32, tag="vps")
                        for ct in range(CV_T):
                            nc.tensor.matmul(
                                ps,
                                ckvn[ct][:, ks],
                                vb_w[:, ct, :],
                                start=(ct == 0),
                                stop=(ct == CV_T - 1),
                            )
                        nc.vector.tensor_copy(v_sb[0][:, ks], ps[:, 0:VD])
                        nc.vector.tensor_copy(v_sb[1][:, ks], ps[:, VD : 2 * VD])

                # ---------------- phase C: attention ----------------
                mskp_cm = tc.tile_pool(name="mskp", bufs=1)
                oww_cm = tc.tile_pool(name="oww", bufs=1)
                mskp = mskp_cm.__enter__()
                oww = oww_cm.__enter__()
                with tc.tile_pool(name="pp", bufs=6) as pp, \
                     tc.tile_pool(name="ep", bufs=3) as ep, \
                     tc.tile_pool(name="rvp", bufs=2) as rvp, \
                     tc.tile_pool(name="ostg", bufs=2) as ostg, \
                     tc.tile_pool(name="s_ps", bufs=3, space="PSUM") as s_ps, \
                     tc.tile_pool(name="rs_ps", bufs=2, space="PSUM") as rs_ps, \
                     tc.tile_pool(name="o_ps", bufs=2, space="PSUM") as o_ps, \
                     tc.tile_pool(name="out_ps", bufs=1, space="PSUM") as out_ps:
                    mask_s = mskp.tile([128, 1024], BF, tag="mask")
                    nc.sync.dma_start(out=mask_s, in_=maskb[:, :])
                    ow_t = oww.tile([128, HPC, D], BF, tag="oww")
                    nc.sync.dma_start(out=ow_t[:, :, :], in_=ow3[:, :, :])
                    for c in range(NCHUNK):
                        cs = slice(NQ * c, NQ * (c + 1))
                        nkt = 4 * (c + 1)
                        for h in range(HPC):
                            kper_h = kperLo if h == 0 else kperHi
                            rs = rs_ps.tile([128, NQ], F32, tag="rs")
                            op = o_ps.tile([128, NQ], F32, tag="op")
                            for kt in range(nkt):
                                ks = slice(128 * kt, 128 * (kt + 1))
                                i = kt - 4 * c
                                lo = 128 * i if i > 0 else 0  # valid q-subrange start
                                qs = slice(NQ * c + lo, NQ * (c + 1))
                                vs = slice(lo, NQ)
                                sp = s_ps.tile([128, NQ], F32, tag="sp")
                                nc.tensor.matmul(
                                    sp[:, vs], kn_T[h][:, ks], qn_T[h][:, qs],
                                    start=True, stop=False,
                                )
                                nc.tensor.matmul(
                                    sp[:, vs], kper_h[:, ks], qpeP[:, qs],
                                    start=False, stop=True,
                                )
                                p_t = pp.tile([128, NQ], BF, tag="p")
                                if kt >= 4 * c:
                                    e_t = ep.tile([128, NQ], BF, tag="e")
                                    nc.scalar.activation(out=e_t[:, vs], in_=sp[:, vs], func=AF.Exp)
                                    nc.vector.tensor_mul(
                                        p_t[:, vs], e_t[:, vs],
                                        mask_s[:, 384 : 896 - lo],
                                    )
                                else:
                                    nc.scalar.activation(out=p_t[:, vs], in_=sp[:, vs], func=AF.Exp)
                                nc.tensor.matmul(
                                    rs[:, vs], ones_t, p_t[:, vs],
                                    start=(kt == 0), stop=(kt == nkt - 1),
                                )
                                nc.tensor.matmul(
                                    op[:, vs],
                                    v_sb[h][:, ks],
                                    p_t[:, vs],
                                    start=(kt == 0), stop=(kt == nkt - 1),
                                )
                            rv = rvp.tile([128, NQ], F32, tag="rv")
                            nc.vector.reciprocal(rv, rs)
                            nc.vector.tensor_mul(o_T[h][:, cs], op, rv)
                        # o_proj for this chunk's 4 s-tiles (both heads now done;
                        # last chunk handled in a post-phase with deeper PSUM)
                        for si in range(4 * c, 4 * (c + 1) if c < NCHUNK - 1 else 4 * c):
                            ss = slice(128 * si, 128 * (si + 1))
                            stg = ostg.tile([128, D], F32, tag="ostg")
                            for nch in range(NCHUNK):
                                ns = slice(NQ * nch, NQ * (nch + 1))
                                ps = out_ps.tile([128, NQ], F32, tag="outps")
                                for j in range(HPC):
                                    nc.tensor.matmul(
                                        ps,
                                        o_T[j][:, ss],
                                        ow_t[:, j, ns],
                                        start=(j == 0),
                                        stop=(j == HPC - 1),
                                    )
                                nc.scalar.activation(out=stg[:, ns], in_=ps, func=AF.Copy)
                            nc.sync.dma_start(out=out[ss, :], in_=stg)
                # ---------------- final chunk o_proj ----------------
                with tc.tile_pool(name="ostg2", bufs=2) as ostg2, \
                     tc.tile_pool(name="out2_ps", bufs=4, space="PSUM") as out2_ps:
                    for si in range(4 * (NCHUNK - 1), 4 * NCHUNK):
                        ss = slice(128 * si, 128 * (si + 1))
                        stg = ostg2.tile([128, D], F32, tag="ostg2")
                        for nch in range(NCHUNK):
                            ns = slice(NQ * nch, NQ * (nch + 1))
                            ps = out2_ps.tile([128, NQ], F32, tag="out2ps")
                            for j in range(HPC):
                                nc.tensor.matmul(
                                    ps,
                                    o_T[j][:, ss],
                                    ow_t[:, j, ns],
                                    start=(j == 0),
                                    stop=(j == HPC - 1),
                                )
                            nc.scalar.activation(out=stg[:, ns], in_=ps, func=AF.Copy)
                        nc.sync.dma_start(out=out[ss, :], in_=stg)
                oww_cm.__exit__(None, None, None)
                mskp_cm.__exit__(None, None, None)

    _split_waits(nc)
    return nc


# ----------------------------------------------------------------------------
# entry point
# ----------------------------------------------------------------------------

def kernel(**inputs):
    global LAST_RESULTS
    shared, per_core = _prep_inputs(inputs)
    if "nc" not in _CACHE:
        _CACHE["nc"] = _build_nc()
    nc = _CACHE["nc"]
    in_maps = []
    for c in range(N_CORES):
        m = {
            "h3": shared["h3"],
            "qa3": shared["qa3"],
            "kva3": shared["kva3"],
            "cosb": shared["cosb"],
            "sinb": shared["sinb"],
            "maskb": shared["maskb"],
            "qf3": per_core[c]["qf3"],
            "kb3": per_core[c]["kb3"],
            "vb3": per_core[c]["vb3"],
            "ow3": per_core[c]["ow3"],
            "hb3": per_core[c]["hb3"],
            "cosk": per_core[c]["cosk"],
            "sink": per_core[c]["sink"],
        }
        in_maps.append(m)
    res = run_bass_kernel_spmd(nc, in_maps, core_ids=list(range(N_CORES)))
    LAST_RESULTS = res
    out = np.zeros((S, D), dtype=np.float32)
    for r in res.results:
        out += r["out"]
    return out.reshape(B, S, D)
